# revision 19
# baseline (speedup 1.0000x reference)
"""GCN encoder/decoder (gnn_message_passing) Trainium2 kernel.

Pull-model with PE segment-sum aggregation:
  - nodes partitioned across 8 cores (owner-computes on dst)
  - per conv: AllGather fp16 feature table; tokens (edges incl self loops)
    sorted by dst 128-block and grouped by table quarter (so dma_gather
    indices fit int16); per 128-token chunk, gather src rows and
    segment-sum them on the PE: psum[feat, seg] += msg[tok, feat]^T @
    S[tok, seg] with S built on-chip (iota==segid), accumulating per
    dst-block in PSUM.  No scatter-add, no HBM round trip for y.
  - dinv[dst] applied during PSUM eviction (broadcast multiply), weight
    GEMM from fp16 staging, BN stats fused into eviction (accum_out),
    1KB AllReduce, scalar-engine affine+ReLU, table rebuild (PE
    transpose + dinv[src] scale + fp16 cast).
"""

import math
import os
import time
from contextlib import ExitStack

import numpy as np

CORES = 8
H = 128
EPS = 1e-5


class Cfg:
    def __init__(self, N, depth=9, sblk=8, cap=8, queues=1, scratch=16384,
                 fused_sbuild=False):
        assert N % CORES == 0
        self.N = N
        self.S = N // CORES
        self.NPC = ((self.S + 127) // 128) * 128
        self.NBLK = self.NPC // 128
        self.QROWS = 2 * self.NPC          # table rows per quarter (2 shards)
        assert self.QROWS <= 32767
        self.depth = depth
        self.nconv = 2 * depth + 1
        self.SB = min(sblk, self.NBLK)     # dst blocks per superblock
        self.NSUP = (self.NBLK + self.SB - 1) // self.SB
        self.CAP = cap                     # max chunks per gather call
        self.QUEUES = queues
        self.SCRATCH = scratch
        self.FUSED_SBUILD = fused_sbuild
        assert cap * 128 <= scratch // 16


FULL = Cfg(100000, fused_sbuild=True)


# ----------------------------------------------------------------------------
# Host-side preprocessing (sharding / token planning)
# ----------------------------------------------------------------------------

def wrap16(a):
    # token i -> [i % 16, i // 16], replicated to 128 partitions
    b = a.astype(np.int16).reshape(-1, 16).T.copy()
    return np.tile(b, (8, 1))


def preprocess(x, edge_index, cfg):
    N, S, NPC, QROWS, NBLK, SB, CAP = (cfg.N, cfg.S, cfg.NPC, cfg.QROWS,
                                       cfg.NBLK, cfg.SB, cfg.CAP)
    src = np.concatenate([np.asarray(edge_index[0], np.int64),
                          np.arange(N, dtype=np.int64)])
    dst = np.concatenate([np.asarray(edge_index[1], np.int64),
                          np.arange(N, dtype=np.int64)])
    deg = np.bincount(dst, minlength=N).astype(np.float32)
    dinv = np.where(deg > 0, 1.0 / np.sqrt(deg), 0.0).astype(np.float32)

    gid = (src // S) * NPC + (src % S)     # padded global row id in table
    shard = dst // S
    NG = 4 * NBLK                          # (quarter, block) groups

    per_core = []
    cnt = np.zeros((CORES, NG), np.int64)
    for k in range(CORES):
        m = shard == k
        g = gid[m]
        d = dst[m] - k * S
        q = g // QROWS
        lrow = g % QROWS
        b = d // 128
        key = q * NBLK + b
        order = np.argsort(key, kind="stable")
        per_core.append((lrow[order], (d % 128)[order]))
        bounds = np.searchsorted(key[order], np.arange(NG + 1))
        per_core[k] = per_core[k] + (bounds,)
        cnt[k] = np.diff(bounds)

    nch = ((cnt + 127) // 128).max(axis=0)             # [NG] static plan
    for b in range(NBLK):                              # every block >=1 chunk
        if nch[b::NBLK].sum() == 0:
            nch[b] = 1

    # PSUM accumulation groups are per 2KB bank = 4 dst blocks; start/stop
    # flags must be one per bank (start marks the whole bank pending-zero).
    total_per_block = nch.reshape(4, NBLK).sum(axis=0)
    NBANK = (NBLK + 3) // 4
    bank_total = np.zeros(NBANK, np.int64)
    for b in range(NBLK):
        bank_total[b // 4] += total_per_block[b]
    bank_seen = np.zeros(NBANK, np.int64)
    plan = []          # [sb] -> list of calls
    segcol = 0
    off16 = 0
    for sbi in range(cfg.NSUP):
        blocks = list(range(sbi * SB, min((sbi + 1) * SB, NBLK)))
        sb_calls = []
        for q in range(4):
            chunk_ids = [(b, i) for b in blocks
                         for i in range(int(nch[q * NBLK + b]))]
            pos = 0
            while pos < len(chunk_ids):
                take = chunk_ids[pos:pos + CAP]
                descs = []
                for jslot, (b, i) in enumerate(take):
                    bk = b // 4
                    first = bank_seen[bk] == 0
                    bank_seen[bk] += 1
                    last = bank_seen[bk] == bank_total[bk]
                    descs.append((jslot, b - sbi * SB, segcol, bool(first),
                                  bool(last)))
                    segcol += 1
                sb_calls.append(dict(q=q, chunks=take, n=len(take),
                                     off16=off16, descs=descs))
                off16 += len(take) * 8
                pos += len(take)
        plan.append(sb_calls)
    nchk = segcol

    in_maps = []
    for k in range(CORES):
        lrow_k, seg_k, bounds = per_core[k]
        idx_cols, seg_cols = [], []
        for sb_calls in plan:
            for call in sb_calls:
                q = call["q"]
                L, Sg = [], []
                for (b, i) in call["chunks"]:
                    gq = q * NBLK + b
                    lo, hi = int(bounds[gq]), int(bounds[gq + 1])
                    s0 = lo + i * 128
                    rows = np.zeros(128, np.int64)
                    segs = np.full(128, -1, np.int64)
                    n = max(0, min(hi - s0, 128))
                    if n > 0:
                        rows[:n] = lrow_k[s0:s0 + n]
                        segs[:n] = seg_k[s0:s0 + n]
                    L.append(rows)
                    Sg.append(segs)
                idx_cols.append(wrap16(np.concatenate(L)))
                seg_cols.append(np.stack(Sg))
        IDX = np.concatenate(idx_cols, axis=1)
        SEGID = np.ascontiguousarray(
            np.concatenate(seg_cols, axis=0).T.astype(np.float32))

        xt = np.zeros((x.shape[1], NPC), dtype=np.float32)
        xt[:, :S] = np.asarray(x[k * S:(k + 1) * S], np.float32).T
        dv = np.zeros(NPC, dtype=np.float32)
        dv[:S] = dinv[k * S:(k + 1) * S]
        dinv_nm = np.ascontiguousarray(dv.reshape(NBLK, 128).T)
        dinvb = np.ascontiguousarray(
            np.broadcast_to(dv, (128, NPC)).astype(np.float16))
        in_maps.append({"xT": xt, "gidx": IDX, "segid": SEGID,
                        "dinv_nm": dinv_nm, "dinvb": dinvb})
    return in_maps, plan, nchk


# ----------------------------------------------------------------------------
# Device kernel
# ----------------------------------------------------------------------------

def build_nc(cfg, plan, nchk, d_in):
    import concourse.bacc as bacc
    import concourse.bass as bass
    import concourse.mybir as mybir
    import concourse.tile as tile

    f32 = mybir.dt.float32
    f16 = mybir.dt.float16
    i16 = mybir.dt.int16
    AF = mybir.ActivationFunctionType
    ALU = mybir.AluOpType
    AX = mybir.AxisListType

    NPC, NBLK, SB, QROWS = cfg.NPC, cfg.NBLK, cfg.SB, cfg.QROWS
    depth = cfg.depth
    nconv = cfg.nconv
    TROWS = CORES * NPC
    MAXSLOT = max(c["n"] for sb in plan for c in sb)
    NS = (NPC + 511) // 512
    STATC = 2 * cfg.NSUP + 2

    # descriptor carveout: ring must hold a whole gather call (CAP*128 descs)
    nc = bacc.Bacc("TRN2", target_bir_lowering=False, debug=False,
                   num_devices=CORES,
                   dynamic_dma_scratch_size=cfg.SCRATCH,
                   num_swdge_queues=cfg.QUEUES)

    # ---- I/O ----
    xT_d = nc.dram_tensor("xT", [d_in, NPC], f32, kind="ExternalInput")
    gidx_d = nc.dram_tensor("gidx", [128, nchk * 8], i16, kind="ExternalInput")
    segid_d = nc.dram_tensor("segid", [128, nchk], f32, kind="ExternalInput")
    dinvnm_d = nc.dram_tensor("dinv_nm", [128, NBLK], f32, kind="ExternalInput")
    dinvb_d = nc.dram_tensor("dinvb", [128, NPC], f16, kind="ExternalInput")
    W0_d = nc.dram_tensor("W0", [d_in, H], f32, kind="ExternalInput")
    Ws1_d = nc.dram_tensor("Ws1", [depth, H, H], f16, kind="ExternalInput")
    Ws2_d = nc.dram_tensor("Ws2", [depth - 1, H, H], f16, kind="ExternalInput")
    Wout_d = nc.dram_tensor("Wout", [H, 1], f16, kind="ExternalInput")
    g1_d = nc.dram_tensor("g1T", [H, depth + 1], f32, kind="ExternalInput")
    b1_d = nc.dram_tensor("b1T", [H, depth + 1], f32, kind="ExternalInput")
    g2_d = nc.dram_tensor("g2T", [H, depth - 1], f32, kind="ExternalInput")
    b2_d = nc.dram_tensor("b2T", [H, depth - 1], f32, kind="ExternalInput")
    ident_d = nc.dram_tensor("ident", [128, 128], f32, kind="ExternalInput")
    out_d = nc.dram_tensor("out", [1, NPC], f32, kind="ExternalOutput")

    # ---- internals ----
    tabs = [nc.dram_tensor(f"tab{i}", [TROWS, H], f16, addr_space="Shared")
            for i in range(2)]
    ulocal = nc.dram_tensor("ulocal", [NPC, H], f16)
    stats_in = nc.dram_tensor("stats_in", [128, 2], f32)
    stats_out = nc.dram_tensor("stats_out", [128, 2], f32, addr_space="Shared")
    xs_d = nc.dram_tensor("xs", [depth, 128, NPC], f32)

    rg = [list(range(CORES))]

    with tile.TileContext(nc, num_cores=CORES) as tc, ExitStack() as ctx:
        persist = ctx.enter_context(tc.tile_pool(name="persist", bufs=1))
        msgp = ctx.enter_context(tc.tile_pool(name="msg", bufs=3))
        sp = ctx.enter_context(tc.tile_pool(name="sbld", bufs=2))
        ytp = ctx.enter_context(tc.tile_pool(name="yt", bufs=2))
        stgp = ctx.enter_context(tc.tile_pool(name="stg", bufs=3))
        wp = ctx.enter_context(tc.tile_pool(name="wp", bufs=2))
        skp = ctx.enter_context(tc.tile_pool(name="skp", bufs=3))
        smallp = ctx.enter_context(tc.tile_pool(name="small", bufs=8))
        obp = ctx.enter_context(tc.tile_pool(name="obp", bufs=2))
        accp = ctx.enter_context(tc.tile_pool(name="accp", bufs=2, space="PSUM"))
        pgemm = ctx.enter_context(tc.tile_pool(name="pgemm", bufs=2, space="PSUM"))

        # persistent tiles
        zbuf = persist.tile([128, NPC], f32)
        idx_sb = persist.tile([128, nchk * 8], i16)
        segid_sb = persist.tile([128, nchk], f32)
        dinvb_sb = persist.tile([128, NPC], f16)
        dinvnm_sb = persist.tile([128, NBLK], f32)
        iota_sb = persist.tile([128, 128], f32)
        ident_sb = persist.tile([128, 128], f32)
        sums_sb = persist.tile([128, STATC], f32)
        sumsq_sb = persist.tile([128, STATC], f32)
        stat2_sb = persist.tile([128, 2], f32)
        sqscr = persist.tile([128, 512], f32)
        wout_sb = persist.tile([128, 1], f16)

        # load persistent data (split large loads across DMA queues)
        PIECE = 8192 * 2  # int16 elems per partition-row piece
        tot16 = nchk * 8
        o = 0
        while o < tot16:
            w = min(PIECE, tot16 - o)
            nc.sync.dma_start(out=idx_sb[:, o:o + w], in_=gidx_d[:, o:o + w])
            o += w
        nc.sync.dma_start(out=segid_sb[:], in_=segid_d[:])
        o = 0
        while o < NPC:
            w = min(4096, NPC - o)
            nc.sync.dma_start(out=dinvb_sb[:, o:o + w], in_=dinvb_d[:, o:o + w])
            o += w
        nc.sync.dma_start(out=dinvnm_sb[:], in_=dinvnm_d[:])
        nc.sync.dma_start(out=ident_sb[:], in_=ident_d[:])
        nc.sync.dma_start(out=wout_sb[:], in_=Wout_d[:])
        nc.gpsimd.iota(iota_sb[:], pattern=[[1, 128]], base=0,
                       channel_multiplier=0,
                       allow_small_or_imprecise_dtypes=True)

        def gemm_weight(t):
            if t == 1 or t == nconv:
                return None
            w = wp.tile([128, 128], f16, tag="w")
            if t <= depth + 1:
                nc.sync.dma_start(out=w[:], in_=Ws1_d[t - 2])
            else:
                nc.sync.dma_start(out=w[:], in_=Ws2_d[t - depth - 2])
            return w

        def bn_params(t):
            gt = smallp.tile([128, 1], f32, tag="gt")
            bt = smallp.tile([128, 1], f32, tag="bt")
            if t <= depth + 1:
                nc.sync.dma_start(out=gt[:], in_=g1_d[:, t - 1:t])
                nc.sync.dma_start(out=bt[:], in_=b1_d[:, t - 1:t])
            else:
                i = t - depth - 2
                nc.sync.dma_start(out=gt[:], in_=g2_d[:, i:i + 1])
                nc.sync.dma_start(out=bt[:], in_=b2_d[:, i:i + 1])
            return gt, bt

        def build_table(t):
            # zbuf (feature-major fp32) -> transpose -> dinv[src] -> fp16
            NB4 = (NBLK + 3) // 4
            for g in range(NB4):
                b0 = 4 * g
                nb = min(4, NBLK - b0)
                st = stgp.tile([128, 4, H], f16, tag="st")
                pt = pgemm.tile([128, 512], f32, tag="pg", name="pt")
                for j in range(nb):
                    b = b0 + j
                    nc.tensor.transpose(
                        pt[:, j * 128:(j + 1) * 128],
                        zbuf[:, b * 128:(b + 1) * 128], ident_sb[:])
                    nc.vector.tensor_scalar_mul(
                        st[:, j, :], pt[:, j * 128:(j + 1) * 128],
                        dinvnm_sb[:, b:b + 1])
                nc.sync.dma_start(
                    out=ulocal[b0 * 128:(b0 + nb) * 128, :]
                    .rearrange("(a p) f -> p a f", p=128),
                    in_=st[:, :nb, :])
            nc.gpsimd.collective_compute(
                "AllGather", ALU.bypass, replica_groups=rg,
                ins=[ulocal[:, :]], outs=[tabs[t % 2][:, :]])

        # ---- stage 0: z0.T = W0.T @ xT ----
        w0 = persist.tile([d_in, H], f32)
        nc.sync.dma_start(out=w0[:], in_=W0_d[:])
        for s in range(NS):
            c0 = s * 512
            cw = min(512, NPC - c0)
            xt = skp.tile([d_in, 512], f32, tag="xt")
            nc.sync.dma_start(out=xt[:, :cw], in_=xT_d[:, c0:c0 + cw])
            pg = pgemm.tile([128, 512], f32, tag="pg")
            nc.tensor.matmul(pg[:, :cw], w0[:], xt[:, :cw],
                             start=True, stop=True)
            nc.scalar.copy(zbuf[:, c0:c0 + cw], pg[:, :cw])
        build_table(0)

        # ---- conv layers ----
        qrr = 0
        for t in range(1, nconv + 1):
            tab = tabs[(t - 1) % 2]
            w = gemm_weight(t)
            scol = 0
            for sbi in range(cfg.NSUP):
                nb_sb = min(SB, NBLK - sbi * SB)
                nacct = (nb_sb + 3) // 4
                acct = [accp.tile([128, 512], f32, tag=f"acct{i}",
                                  name=f"acct{i}")
                        for i in range(nacct)]
                accs = [acct[j // 4][:, (j % 4) * 128:(j % 4 + 1) * 128]
                        for j in range(nb_sb)]
                for call in plan[sbi]:
                    q, ncall, off16 = call["q"], call["n"], call["off16"]
                    msg = msgp.tile([128, MAXSLOT, H], f16, tag="msg")
                    nc.gpsimd.dma_gather(
                        msg[:, :ncall, :],
                        tab[q * QROWS:(q + 1) * QROWS, :],
                        idx_sb[:, off16:off16 + ncall * 8],
                        ncall * 128, ncall * 128, H,
                        queue_num=qrr % cfg.QUEUES)
                    qrr += 1
                    c0 = call["descs"][0][2]
                    st_ = sp.tile([128, MAXSLOT, 128], f16, tag="S")
                    if cfg.FUSED_SBUILD:
                        nc.vector.tensor_tensor(
                            st_[:, :ncall, :],
                            iota_sb[:].unsqueeze(1)
                            .broadcast_to([128, ncall, 128]),
                            segid_sb[:, c0:c0 + ncall].unsqueeze(2)
                            .broadcast_to([128, ncall, 128]),
                            op=ALU.is_equal)
                    else:
                        for jj in range(ncall):
                            nc.vector.tensor_scalar(
                                st_[:, jj, :], iota_sb[:],
                                segid_sb[:, c0 + jj:c0 + jj + 1], None,
                                op0=ALU.is_equal)
                    for (jslot, jacc, segc, first, last) in call["descs"]:
                        nc.tensor.matmul(
                            accs[jacc], msg[:, jslot, :], st_[:, jslot, :],
                            start=first, stop=last)

                # ---- evict superblock ----
                nb0 = sbi * SB * 128
                if t == 1:
                    for j in range(nb_sb):
                        nc.vector.tensor_mul(
                            zbuf[:, nb0 + j * 128: nb0 + (j + 1) * 128],
                            accs[j],
                            dinvb_sb[:, nb0 + j * 128: nb0 + (j + 1) * 128])
                    continue
                ytmp = ytp.tile([128, SB * 128], f16, tag="ytmp")
                for j in range(nb_sb):
                    nc.vector.tensor_mul(
                        ytmp[:, j * 128:(j + 1) * 128], accs[j],
                        dinvb_sb[:, nb0 + j * 128: nb0 + (j + 1) * 128])
                for hw_ in range(0, nb_sb * 128, 512):
                    cw = min(512, nb_sb * 128 - hw_)
                    if t < nconv:
                        pg = pgemm.tile([128, 512], f32, tag="pg")
                        nc.tensor.matmul(pg[:, :cw], w[:], ytmp[:, hw_:hw_ + cw],
                                         start=True, stop=True)
                        nc.scalar.activation(
                            zbuf[:, nb0 + hw_: nb0 + hw_ + cw], pg[:, :cw],
                            AF.Copy, accum_out=sums_sb[:, scol:scol + 1])
                        nc.scalar.activation(
                            sqscr[:, :cw], pg[:, :cw],
                            AF.Square, accum_out=sumsq_sb[:, scol:scol + 1])
                        scol += 1
                    else:
                        po = pgemm.tile([128, 512], f32, tag="pg", name="po")
                        nc.tensor.matmul(po[0:1, :cw], wout_sb[:],
                                         ytmp[:, hw_:hw_ + cw],
                                         start=True, stop=True)
                        ob = obp.tile([1, 512], f32, tag="ob")
                        nc.scalar.activation(ob[:, :cw], po[0:1, :cw],
                                             AF.Sigmoid)
                        nc.sync.dma_start(
                            out=out_d[:, nb0 + hw_: nb0 + hw_ + cw],
                            in_=ob[:, :cw])

            if t == nconv:
                break

            # ---- BN stats ----
            if t == 1:
                for s in range(NS):
                    c0 = s * 512
                    cw = min(512, NPC - c0)
                    zsl = zbuf[:, c0:c0 + cw]
                    nc.vector.tensor_reduce(sums_sb[:, s:s + 1], zsl,
                                            axis=AX.X, op=ALU.add)
                    nc.vector.tensor_mul(sqscr[:, :cw], zsl, zsl)
                    nc.vector.tensor_reduce(sumsq_sb[:, s:s + 1], sqscr[:, :cw],
                                            axis=AX.X, op=ALU.add)
                scol = NS
            nc.vector.tensor_reduce(stat2_sb[:, 0:1], sums_sb[:, :scol],
                                    axis=AX.X, op=ALU.add)
            nc.vector.tensor_reduce(stat2_sb[:, 1:2], sumsq_sb[:, :scol],
                                    axis=AX.X, op=ALU.add)
            nc.sync.dma_start(out=stats_in[:, :], in_=stat2_sb[:])
            nc.gpsimd.collective_compute(
                "AllReduce", ALU.add, replica_groups=rg,
                ins=[stats_in[:, :]], outs=[stats_out[:, :]])
            gst = smallp.tile([128, 2], f32, tag="gst")
            nc.sync.dma_start(out=gst[:], in_=stats_out[:, :])

            mean = smallp.tile([128, 1], f32, tag="mean")
            m2 = smallp.tile([128, 1], f32, tag="m2")
            var = smallp.tile([128, 1], f32, tag="var")
            scl = smallp.tile([128, 1], f32, tag="scl")
            sft = smallp.tile([128, 1], f32, tag="sft")
            inv_n = 1.0 / float(cfg.N)
            nc.vector.tensor_scalar_mul(mean[:], gst[:, 0:1], inv_n)
            nc.vector.tensor_scalar_mul(var[:], gst[:, 1:2], inv_n)
            nc.vector.tensor_mul(m2[:], mean[:], mean[:])
            nc.vector.scalar_tensor_tensor(
                var[:], m2[:], -1.0, var[:], op0=ALU.mult, op1=ALU.add)
            nc.vector.tensor_scalar_add(var[:], var[:], EPS)
            gt, bt = bn_params(t)
            nc.scalar.sqrt(scl[:], var[:])
            nc.vector.reciprocal(scl[:], scl[:])
            nc.vector.tensor_mul(scl[:], scl[:], gt[:])
            nc.vector.tensor_mul(sft[:], mean[:], scl[:])
            nc.vector.scalar_tensor_tensor(
                sft[:], sft[:], -1.0, bt[:], op0=ALU.mult, op1=ALU.add)

            # ---- normalize + relu (in place on zbuf) ----
            for s in range(NS):
                c0 = s * 512
                cw = min(512, NPC - c0)
                nc.scalar.activation(zbuf[:, c0:c0 + cw], zbuf[:, c0:c0 + cw],
                                     AF.Relu, bias=sft[:], scale=scl[:])

            # ---- stash xs / skip add ----
            if t <= depth:
                for s in range(NS):
                    c0 = s * 512
                    cw = min(512, NPC - c0)
                    nc.sync.dma_start(out=xs_d[t - 1][:, c0:c0 + cw],
                                      in_=zbuf[:, c0:c0 + cw])
            if t + 1 >= depth + 2:
                j = 2 * depth - t
                for s in range(NS):
                    c0 = s * 512
                    cw = min(512, NPC - c0)
                    sk = skp.tile([128, 512], f32, tag="sk")
                    nc.sync.dma_start(out=sk[:, :cw], in_=xs_d[j][:, c0:c0 + cw])
                    nc.vector.tensor_add(zbuf[:, c0:c0 + cw],
                                         zbuf[:, c0:c0 + cw], sk[:, :cw])

            build_table(t)

    nc.compile()
    return nc


# ----------------------------------------------------------------------------
# Entry point
# ----------------------------------------------------------------------------

LAST_INFO = {}


def _run(inputs, cfg):
    from concourse.bass_utils import run_bass_kernel_spmd

    x = np.asarray(inputs["x"], np.float32)
    d_in = x.shape[1]
    t0 = time.time()
    shard_maps, plan, nchk = preprocess(x, inputs["edge_index"], cfg)
    t1 = time.time()
    nc = build_nc(cfg, plan, nchk, d_in)
    t2 = time.time()

    common = {
        "W0": np.asarray(inputs["W0"], np.float32),
        "Ws1": np.asarray(inputs["Ws1"], np.float16),
        "Ws2": np.asarray(inputs["Ws2"], np.float16),
        "Wout": np.asarray(inputs["Wout"], np.float16),
        "g1T": np.ascontiguousarray(np.asarray(inputs["g1"], np.float32).T),
        "b1T": np.ascontiguousarray(np.asarray(inputs["b1"], np.float32).T),
        "g2T": np.ascontiguousarray(np.asarray(inputs["g2"], np.float32).T),
        "b2T": np.ascontiguousarray(np.asarray(inputs["b2"], np.float32).T),
        "ident": np.eye(128, dtype=np.float32),
    }
    in_maps = [dict(m, **common) for m in shard_maps]
    kw = {}
    if os.environ.get("KBENCH_TRACE"):
        kw = dict(trace=True, tmpdir=os.environ.get("KBENCH_TMPDIR") or None)
    res = run_bass_kernel_spmd(nc, in_maps, list(range(CORES)), **kw)
    t3 = time.time()
    LAST_INFO.update(preprocess_s=t1 - t0, build_s=t2 - t1, run_s=t3 - t2,
                     exec_time_ns=res.exec_time_ns)
    out = np.concatenate(
        [res.results[k]["out"][0, :cfg.S] for k in range(CORES)])
    return out.reshape(cfg.N, 1).astype(np.float32)


def kernel(**inputs):
    return _run(inputs, FULL)


# revision 26
# speedup vs baseline: 1.5559x; 1.5559x over previous
"""GCN encoder/decoder (gnn_message_passing) Trainium2 kernel.

Pull-model with PE segment-sum aggregation:
  - nodes partitioned across 8 cores (owner-computes on dst)
  - per conv: AllGather fp16 feature table; tokens (edges incl self loops)
    sorted by dst 128-block and grouped by table quarter (so dma_gather
    indices fit int16); per 128-token chunk, gather src rows and
    segment-sum them on the PE: psum[feat, seg] += msg[tok, feat]^T @
    S[tok, seg] with S built on-chip (iota==segid), accumulating per
    dst-block in PSUM.  No scatter-add, no HBM round trip for y.
  - dinv[dst] applied during PSUM eviction (broadcast multiply), weight
    GEMM from fp16 staging, BN stats fused into eviction (accum_out),
    1KB AllReduce, scalar-engine affine+ReLU, table rebuild (PE
    transpose + dinv[src] scale + fp16 cast).
"""

import math
import os
import time
from contextlib import ExitStack

import numpy as np

CORES = 8
H = 128
EPS = 1e-5


class Cfg:
    def __init__(self, N, depth=9, sblk=8, cap=8, queues=1, scratch=16384,
                 fused_sbuild=False):
        assert N % CORES == 0
        self.N = N
        self.S = N // CORES
        self.NPC = ((self.S + 127) // 128) * 128
        self.NBLK = self.NPC // 128
        self.QROWS = 2 * self.NPC          # table rows per quarter (2 shards)
        assert self.QROWS <= 32767
        self.depth = depth
        self.nconv = 2 * depth + 1
        self.SB = min(sblk, self.NBLK)     # dst blocks per superblock
        self.NSUP = (self.NBLK + self.SB - 1) // self.SB
        self.CAP = cap                     # max chunks per gather call
        self.QUEUES = queues
        self.SCRATCH = scratch
        self.FUSED_SBUILD = fused_sbuild
        assert cap * 128 <= scratch // 16


FULL = Cfg(100000, fused_sbuild=True, queues=4)


# ----------------------------------------------------------------------------
# Host-side preprocessing (sharding / token planning)
# ----------------------------------------------------------------------------

def wrap16(a):
    # token i -> [i % 16, i // 16], replicated to 128 partitions
    b = a.astype(np.int16).reshape(-1, 16).T.copy()
    return np.tile(b, (8, 1))


def preprocess(x, edge_index, cfg):
    N, S, NPC, QROWS, NBLK, SB, CAP = (cfg.N, cfg.S, cfg.NPC, cfg.QROWS,
                                       cfg.NBLK, cfg.SB, cfg.CAP)
    # self loops are folded into eviction as dinv^2 * z_prev (no tokens)
    src = np.asarray(edge_index[0], np.int64)
    dst = np.asarray(edge_index[1], np.int64)
    deg = np.bincount(dst, minlength=N).astype(np.float32) + 1.0
    dinv = (1.0 / np.sqrt(deg)).astype(np.float32)

    gid = (src // S) * NPC + (src % S)     # padded global row id in table
    shard = dst // S
    NG = 4 * NBLK                          # (quarter, block) groups

    per_core = []
    cnt = np.zeros((CORES, NG), np.int64)
    for k in range(CORES):
        m = shard == k
        g = gid[m]
        d = dst[m] - k * S
        q = g // QROWS
        lrow = g % QROWS
        b = d // 128
        key = q * NBLK + b
        order = np.argsort(key, kind="stable")
        per_core.append((lrow[order], (d % 128)[order]))
        bounds = np.searchsorted(key[order], np.arange(NG + 1))
        per_core[k] = per_core[k] + (bounds,)
        cnt[k] = np.diff(bounds)

    nch = ((cnt + 127) // 128).max(axis=0)             # [NG] static plan
    for b in range(NBLK):                              # every block >=1 chunk
        if nch[b::NBLK].sum() == 0:
            nch[b] = 1

    # PSUM accumulation groups are per 2KB bank = 4 dst blocks; start/stop
    # flags must be one per bank (start marks the whole bank pending-zero).
    total_per_block = nch.reshape(4, NBLK).sum(axis=0)
    NBANK = (NBLK + 3) // 4
    bank_total = np.zeros(NBANK, np.int64)
    for b in range(NBLK):
        bank_total[b // 4] += total_per_block[b]
    bank_seen = np.zeros(NBANK, np.int64)
    plan = []          # [sb] -> list of calls
    segcol = 0
    off16 = 0
    for sbi in range(cfg.NSUP):
        blocks = list(range(sbi * SB, min((sbi + 1) * SB, NBLK)))
        sb_calls = []
        for q in range(4):
            chunk_ids = [(b, i) for b in blocks
                         for i in range(int(nch[q * NBLK + b]))]
            pos = 0
            while pos < len(chunk_ids):
                take = chunk_ids[pos:pos + CAP]
                descs = []
                for jslot, (b, i) in enumerate(take):
                    bk = b // 4
                    first = bank_seen[bk] == 0
                    bank_seen[bk] += 1
                    last = bank_seen[bk] == bank_total[bk]
                    descs.append((jslot, b - sbi * SB, segcol, bool(first),
                                  bool(last)))
                    segcol += 1
                sb_calls.append(dict(q=q, chunks=take, n=len(take),
                                     off16=off16, descs=descs))
                off16 += len(take) * 8
                pos += len(take)
        plan.append(sb_calls)
    nchk = segcol

    in_maps = []
    for k in range(CORES):
        lrow_k, seg_k, bounds = per_core[k]
        idx_cols, seg_cols = [], []
        for sb_calls in plan:
            for call in sb_calls:
                q = call["q"]
                L, Sg = [], []
                for (b, i) in call["chunks"]:
                    gq = q * NBLK + b
                    lo, hi = int(bounds[gq]), int(bounds[gq + 1])
                    s0 = lo + i * 128
                    rows = np.zeros(128, np.int64)
                    segs = np.full(128, -1, np.int64)
                    n = max(0, min(hi - s0, 128))
                    if n > 0:
                        rows[:n] = lrow_k[s0:s0 + n]
                        segs[:n] = seg_k[s0:s0 + n]
                    L.append(rows)
                    Sg.append(segs)
                idx_cols.append(wrap16(np.concatenate(L)))
                seg_cols.append(np.stack(Sg))
        IDX = np.concatenate(idx_cols, axis=1)
        SEGID = np.ascontiguousarray(
            np.concatenate(seg_cols, axis=0).T.astype(np.float32))

        xt = np.zeros((x.shape[1], NPC), dtype=np.float32)
        xt[:, :S] = np.asarray(x[k * S:(k + 1) * S], np.float32).T
        dv = np.zeros(NPC, dtype=np.float32)
        dv[:S] = dinv[k * S:(k + 1) * S]
        dinv_nm = np.ascontiguousarray(dv.reshape(NBLK, 128).T)
        dinvb = np.ascontiguousarray(
            np.broadcast_to(dv, (128, NPC)).astype(np.float16))
        dinv2b = np.ascontiguousarray(
            np.broadcast_to(dv * dv, (128, NPC)).astype(np.float16))
        in_maps.append({"xT": xt, "gidx": IDX, "segid": SEGID,
                        "dinv_nm": dinv_nm, "dinvb": dinvb,
                        "dinv2b": dinv2b})
    return in_maps, plan, nchk


# ----------------------------------------------------------------------------
# Device kernel
# ----------------------------------------------------------------------------

def build_nc(cfg, plan, nchk, d_in):
    import concourse.bacc as bacc
    import concourse.bass as bass
    import concourse.mybir as mybir
    import concourse.tile as tile

    f32 = mybir.dt.float32
    f16 = mybir.dt.float16
    i16 = mybir.dt.int16
    AF = mybir.ActivationFunctionType
    ALU = mybir.AluOpType
    AX = mybir.AxisListType

    NPC, NBLK, SB, QROWS = cfg.NPC, cfg.NBLK, cfg.SB, cfg.QROWS
    depth = cfg.depth
    nconv = cfg.nconv
    TROWS = CORES * NPC
    MAXSLOT = max(c["n"] for sb in plan for c in sb)
    NS = (NPC + 511) // 512
    STATC = 2 * cfg.NSUP + 2

    # descriptor carveout: ring must hold a whole gather call (CAP*128 descs)
    nc = bacc.Bacc("TRN2", target_bir_lowering=False, debug=False,
                   num_devices=CORES,
                   dynamic_dma_scratch_size=cfg.SCRATCH,
                   num_swdge_queues=cfg.QUEUES)

    # ---- I/O ----
    xT_d = nc.dram_tensor("xT", [d_in, NPC], f32, kind="ExternalInput")
    gidx_d = nc.dram_tensor("gidx", [128, nchk * 8], i16, kind="ExternalInput")
    segid_d = nc.dram_tensor("segid", [128, nchk], f32, kind="ExternalInput")
    dinvnm_d = nc.dram_tensor("dinv_nm", [128, NBLK], f32, kind="ExternalInput")
    dinvb_d = nc.dram_tensor("dinvb", [128, NPC], f16, kind="ExternalInput")
    dinv2b_d = nc.dram_tensor("dinv2b", [128, NPC], f16, kind="ExternalInput")
    W0_d = nc.dram_tensor("W0", [d_in, H], f32, kind="ExternalInput")
    Ws1_d = nc.dram_tensor("Ws1", [depth, H, H], f16, kind="ExternalInput")
    Ws2_d = nc.dram_tensor("Ws2", [depth - 1, H, H], f16, kind="ExternalInput")
    Wout_d = nc.dram_tensor("Wout", [H, 1], f16, kind="ExternalInput")
    g1_d = nc.dram_tensor("g1T", [H, depth + 1], f32, kind="ExternalInput")
    b1_d = nc.dram_tensor("b1T", [H, depth + 1], f32, kind="ExternalInput")
    g2_d = nc.dram_tensor("g2T", [H, depth - 1], f32, kind="ExternalInput")
    b2_d = nc.dram_tensor("b2T", [H, depth - 1], f32, kind="ExternalInput")
    ident_d = nc.dram_tensor("ident", [128, 128], f32, kind="ExternalInput")
    out_d = nc.dram_tensor("out", [1, NPC], f32, kind="ExternalOutput")

    # ---- internals ----
    tabs = [nc.dram_tensor(f"tab{i}", [TROWS, H], f16, addr_space="Shared")
            for i in range(2)]
    ulocal = nc.dram_tensor("ulocal", [NPC, H], f16)
    stats_in = nc.dram_tensor("stats_in", [128, 2], f32)
    stats_out = nc.dram_tensor("stats_out", [128, 2], f32, addr_space="Shared")
    xs_d = nc.dram_tensor("xs", [depth, 128, NPC], f32)

    rg = [list(range(CORES))]

    with tile.TileContext(nc, num_cores=CORES) as tc, ExitStack() as ctx:
        persist = ctx.enter_context(tc.tile_pool(name="persist", bufs=1))
        msgp = ctx.enter_context(tc.tile_pool(name="msg", bufs=3))
        sp = ctx.enter_context(tc.tile_pool(name="sbld", bufs=2))
        ytp = ctx.enter_context(tc.tile_pool(name="yt", bufs=2))
        stgp = ctx.enter_context(tc.tile_pool(name="stg", bufs=3))
        wp = ctx.enter_context(tc.tile_pool(name="wp", bufs=2))
        skp = ctx.enter_context(tc.tile_pool(name="skp", bufs=3))
        smallp = ctx.enter_context(tc.tile_pool(name="small", bufs=8))
        obp = ctx.enter_context(tc.tile_pool(name="obp", bufs=2))
        accp = ctx.enter_context(tc.tile_pool(name="accp", bufs=2, space="PSUM"))
        pgemm = ctx.enter_context(tc.tile_pool(name="pgemm", bufs=2, space="PSUM"))

        # persistent tiles
        zbuf = persist.tile([128, NPC], f32)
        idx_sb = persist.tile([128, nchk * 8], i16)
        segid_sb = persist.tile([128, nchk], f32)
        dinvb_sb = persist.tile([128, NPC], f16)
        dinv2b_sb = persist.tile([128, NPC], f16)
        dinvnm_sb = persist.tile([128, NBLK], f32)
        iota_sb = persist.tile([128, 128], f32)
        ident_sb = persist.tile([128, 128], f32)
        sums_sb = persist.tile([128, STATC], f32)
        sumsq_sb = persist.tile([128, STATC], f32)
        stat2_sb = persist.tile([128, 2], f32)
        sqscr = persist.tile([128, 512], f32)
        wout_sb = persist.tile([128, 1], f16)

        # load persistent data (split large loads across DMA queues)
        PIECE = 8192 * 2  # int16 elems per partition-row piece
        tot16 = nchk * 8
        o = 0
        while o < tot16:
            w = min(PIECE, tot16 - o)
            nc.sync.dma_start(out=idx_sb[:, o:o + w], in_=gidx_d[:, o:o + w])
            o += w
        nc.sync.dma_start(out=segid_sb[:], in_=segid_d[:])
        o = 0
        while o < NPC:
            w = min(4096, NPC - o)
            nc.sync.dma_start(out=dinvb_sb[:, o:o + w], in_=dinvb_d[:, o:o + w])
            nc.sync.dma_start(out=dinv2b_sb[:, o:o + w],
                              in_=dinv2b_d[:, o:o + w])
            o += w
        nc.sync.dma_start(out=dinvnm_sb[:], in_=dinvnm_d[:])
        nc.sync.dma_start(out=ident_sb[:], in_=ident_d[:])
        nc.sync.dma_start(out=wout_sb[:], in_=Wout_d[:])
        nc.gpsimd.iota(iota_sb[:], pattern=[[1, 128]], base=0,
                       channel_multiplier=0,
                       allow_small_or_imprecise_dtypes=True)

        def gemm_weight(t):
            if t == 1 or t == nconv:
                return None
            w = wp.tile([128, 128], f16, tag="w")
            if t <= depth + 1:
                nc.sync.dma_start(out=w[:], in_=Ws1_d[t - 2])
            else:
                nc.sync.dma_start(out=w[:], in_=Ws2_d[t - depth - 2])
            return w

        def bn_params(t):
            gt = smallp.tile([128, 1], f32, tag="gt")
            bt = smallp.tile([128, 1], f32, tag="bt")
            if t <= depth + 1:
                nc.sync.dma_start(out=gt[:], in_=g1_d[:, t - 1:t])
                nc.sync.dma_start(out=bt[:], in_=b1_d[:, t - 1:t])
            else:
                i = t - depth - 2
                nc.sync.dma_start(out=gt[:], in_=g2_d[:, i:i + 1])
                nc.sync.dma_start(out=bt[:], in_=b2_d[:, i:i + 1])
            return gt, bt

        def build_table(t):
            # zbuf (feature-major fp32) -> transpose -> dinv[src] -> fp16
            NB4 = (NBLK + 3) // 4
            for g in range(NB4):
                b0 = 4 * g
                nb = min(4, NBLK - b0)
                st = stgp.tile([128, 4, H], f16, tag="st")
                pt = pgemm.tile([128, 512], f32, tag="pg", name="pt")
                for j in range(nb):
                    b = b0 + j
                    nc.tensor.transpose(
                        pt[:, j * 128:(j + 1) * 128],
                        zbuf[:, b * 128:(b + 1) * 128], ident_sb[:])
                    nc.vector.tensor_scalar_mul(
                        st[:, j, :], pt[:, j * 128:(j + 1) * 128],
                        dinvnm_sb[:, b:b + 1])
                nc.sync.dma_start(
                    out=ulocal[b0 * 128:(b0 + nb) * 128, :]
                    .rearrange("(a p) f -> p a f", p=128),
                    in_=st[:, :nb, :])
            nc.gpsimd.collective_compute(
                "AllGather", ALU.bypass, replica_groups=rg,
                ins=[ulocal[:, :]], outs=[tabs[t % 2][:, :]])

        # ---- stage 0: z0.T = W0.T @ xT ----
        w0 = persist.tile([d_in, H], f32)
        nc.sync.dma_start(out=w0[:], in_=W0_d[:])
        for s in range(NS):
            c0 = s * 512
            cw = min(512, NPC - c0)
            xt = skp.tile([d_in, 512], f32, tag="xt")
            nc.sync.dma_start(out=xt[:, :cw], in_=xT_d[:, c0:c0 + cw])
            pg = pgemm.tile([128, 512], f32, tag="pg")
            nc.tensor.matmul(pg[:, :cw], w0[:], xt[:, :cw],
                             start=True, stop=True)
            nc.scalar.copy(zbuf[:, c0:c0 + cw], pg[:, :cw])
        build_table(0)

        # ---- conv layers ----
        qrr = 0
        for t in range(1, nconv + 1):
            tab = tabs[(t - 1) % 2]
            w = gemm_weight(t)
            scol = 0
            for sbi in range(cfg.NSUP):
                nb_sb = min(SB, NBLK - sbi * SB)
                nacct = (nb_sb + 3) // 4
                acct = [accp.tile([128, 512], f32, tag=f"acct{i}",
                                  name=f"acct{i}")
                        for i in range(nacct)]
                accs = [acct[j // 4][:, (j % 4) * 128:(j % 4 + 1) * 128]
                        for j in range(nb_sb)]
                for call in plan[sbi]:
                    q, ncall, off16 = call["q"], call["n"], call["off16"]
                    msg = msgp.tile([128, MAXSLOT, H], f16, tag="msg")
                    nc.gpsimd.dma_gather(
                        msg[:, :ncall, :],
                        tab[q * QROWS:(q + 1) * QROWS, :],
                        idx_sb[:, off16:off16 + ncall * 8],
                        ncall * 128, ncall * 128, H,
                        queue_num=qrr % cfg.QUEUES)
                    qrr += 1
                    c0 = call["descs"][0][2]
                    st_ = sp.tile([128, MAXSLOT, 128], f16, tag="S")
                    if cfg.FUSED_SBUILD:
                        nc.vector.tensor_tensor(
                            st_[:, :ncall, :],
                            iota_sb[:].unsqueeze(1)
                            .broadcast_to([128, ncall, 128]),
                            segid_sb[:, c0:c0 + ncall].unsqueeze(2)
                            .broadcast_to([128, ncall, 128]),
                            op=ALU.is_equal)
                    else:
                        for jj in range(ncall):
                            nc.vector.tensor_scalar(
                                st_[:, jj, :], iota_sb[:],
                                segid_sb[:, c0 + jj:c0 + jj + 1], None,
                                op0=ALU.is_equal)
                    for (jslot, jacc, segc, first, last) in call["descs"]:
                        nc.tensor.matmul(
                            accs[jacc], msg[:, jslot, :], st_[:, jslot, :],
                            start=first, stop=last)

                # ---- evict superblock (y*dinv + self loop dinv^2*z_prev) ----
                nb0 = sbi * SB * 128
                if t == 1:
                    for j in range(nb_sb):
                        cols = slice(nb0 + j * 128, nb0 + (j + 1) * 128)
                        stmp = ytp.tile([128, 128], f16, tag="slf", name="stmp")
                        nc.vector.tensor_mul(stmp[:], zbuf[:, cols],
                                             dinv2b_sb[:, cols])
                        nc.vector.tensor_mul(zbuf[:, cols], accs[j],
                                             dinvb_sb[:, cols])
                        nc.vector.tensor_add(zbuf[:, cols], zbuf[:, cols],
                                             stmp[:])
                    continue
                ytmp = ytp.tile([128, SB * 128], f16, tag="ytmp")
                for j in range(nb_sb):
                    cols = slice(nb0 + j * 128, nb0 + (j + 1) * 128)
                    ycols = slice(j * 128, (j + 1) * 128)
                    stmp = ytp.tile([128, 128], f16, tag="slf", name="stmp")
                    nc.vector.tensor_mul(stmp[:], zbuf[:, cols],
                                         dinv2b_sb[:, cols])
                    nc.vector.tensor_mul(ytmp[:, ycols], accs[j],
                                         dinvb_sb[:, cols])
                    nc.vector.tensor_add(ytmp[:, ycols], ytmp[:, ycols],
                                         stmp[:])
                for hw_ in range(0, nb_sb * 128, 512):
                    cw = min(512, nb_sb * 128 - hw_)
                    if t < nconv:
                        pg = pgemm.tile([128, 512], f32, tag="pg")
                        nc.tensor.matmul(pg[:, :cw], w[:], ytmp[:, hw_:hw_ + cw],
                                         start=True, stop=True)
                        nc.scalar.activation(
                            zbuf[:, nb0 + hw_: nb0 + hw_ + cw], pg[:, :cw],
                            AF.Copy, accum_out=sums_sb[:, scol:scol + 1])
                        nc.scalar.activation(
                            sqscr[:, :cw], pg[:, :cw],
                            AF.Square, accum_out=sumsq_sb[:, scol:scol + 1])
                        scol += 1
                    else:
                        po = pgemm.tile([128, 512], f32, tag="pg", name="po")
                        nc.tensor.matmul(po[0:1, :cw], wout_sb[:],
                                         ytmp[:, hw_:hw_ + cw],
                                         start=True, stop=True)
                        ob = obp.tile([1, 512], f32, tag="ob")
                        nc.scalar.activation(ob[:, :cw], po[0:1, :cw],
                                             AF.Sigmoid)
                        nc.sync.dma_start(
                            out=out_d[:, nb0 + hw_: nb0 + hw_ + cw],
                            in_=ob[:, :cw])

            if t == nconv:
                break

            # ---- BN stats ----
            if t == 1:
                for s in range(NS):
                    c0 = s * 512
                    cw = min(512, NPC - c0)
                    zsl = zbuf[:, c0:c0 + cw]
                    nc.vector.tensor_reduce(sums_sb[:, s:s + 1], zsl,
                                            axis=AX.X, op=ALU.add)
                    nc.vector.tensor_mul(sqscr[:, :cw], zsl, zsl)
                    nc.vector.tensor_reduce(sumsq_sb[:, s:s + 1], sqscr[:, :cw],
                                            axis=AX.X, op=ALU.add)
                scol = NS
            nc.vector.tensor_reduce(stat2_sb[:, 0:1], sums_sb[:, :scol],
                                    axis=AX.X, op=ALU.add)
            nc.vector.tensor_reduce(stat2_sb[:, 1:2], sumsq_sb[:, :scol],
                                    axis=AX.X, op=ALU.add)
            nc.sync.dma_start(out=stats_in[:, :], in_=stat2_sb[:])
            nc.gpsimd.collective_compute(
                "AllReduce", ALU.add, replica_groups=rg,
                ins=[stats_in[:, :]], outs=[stats_out[:, :]])
            gst = smallp.tile([128, 2], f32, tag="gst")
            nc.sync.dma_start(out=gst[:], in_=stats_out[:, :])

            mean = smallp.tile([128, 1], f32, tag="mean")
            m2 = smallp.tile([128, 1], f32, tag="m2")
            var = smallp.tile([128, 1], f32, tag="var")
            scl = smallp.tile([128, 1], f32, tag="scl")
            sft = smallp.tile([128, 1], f32, tag="sft")
            inv_n = 1.0 / float(cfg.N)
            nc.vector.tensor_scalar_mul(mean[:], gst[:, 0:1], inv_n)
            nc.vector.tensor_scalar_mul(var[:], gst[:, 1:2], inv_n)
            nc.vector.tensor_mul(m2[:], mean[:], mean[:])
            nc.vector.scalar_tensor_tensor(
                var[:], m2[:], -1.0, var[:], op0=ALU.mult, op1=ALU.add)
            nc.vector.tensor_scalar_add(var[:], var[:], EPS)
            gt, bt = bn_params(t)
            nc.scalar.sqrt(scl[:], var[:])
            nc.vector.reciprocal(scl[:], scl[:])
            nc.vector.tensor_mul(scl[:], scl[:], gt[:])
            nc.vector.tensor_mul(sft[:], mean[:], scl[:])
            nc.vector.scalar_tensor_tensor(
                sft[:], sft[:], -1.0, bt[:], op0=ALU.mult, op1=ALU.add)

            # ---- normalize + relu (in place on zbuf) ----
            for s in range(NS):
                c0 = s * 512
                cw = min(512, NPC - c0)
                nc.scalar.activation(zbuf[:, c0:c0 + cw], zbuf[:, c0:c0 + cw],
                                     AF.Relu, bias=sft[:], scale=scl[:])

            # ---- stash xs / skip add ----
            if t <= depth:
                for s in range(NS):
                    c0 = s * 512
                    cw = min(512, NPC - c0)
                    nc.sync.dma_start(out=xs_d[t - 1][:, c0:c0 + cw],
                                      in_=zbuf[:, c0:c0 + cw])
            if t + 1 >= depth + 2:
                j = 2 * depth - t
                for s in range(NS):
                    c0 = s * 512
                    cw = min(512, NPC - c0)
                    sk = skp.tile([128, 512], f32, tag="sk")
                    nc.sync.dma_start(out=sk[:, :cw], in_=xs_d[j][:, c0:c0 + cw])
                    nc.vector.tensor_add(zbuf[:, c0:c0 + cw],
                                         zbuf[:, c0:c0 + cw], sk[:, :cw])

            build_table(t)

    nc.compile()
    return nc


# ----------------------------------------------------------------------------
# Entry point
# ----------------------------------------------------------------------------

LAST_INFO = {}


def _run(inputs, cfg):
    from concourse.bass_utils import run_bass_kernel_spmd

    x = np.asarray(inputs["x"], np.float32)
    d_in = x.shape[1]
    t0 = time.time()
    shard_maps, plan, nchk = preprocess(x, inputs["edge_index"], cfg)
    t1 = time.time()
    nc = build_nc(cfg, plan, nchk, d_in)
    t2 = time.time()

    common = {
        "W0": np.asarray(inputs["W0"], np.float32),
        "Ws1": np.asarray(inputs["Ws1"], np.float16),
        "Ws2": np.asarray(inputs["Ws2"], np.float16),
        "Wout": np.asarray(inputs["Wout"], np.float16),
        "g1T": np.ascontiguousarray(np.asarray(inputs["g1"], np.float32).T),
        "b1T": np.ascontiguousarray(np.asarray(inputs["b1"], np.float32).T),
        "g2T": np.ascontiguousarray(np.asarray(inputs["g2"], np.float32).T),
        "b2T": np.ascontiguousarray(np.asarray(inputs["b2"], np.float32).T),
        "ident": np.eye(128, dtype=np.float32),
    }
    in_maps = [dict(m, **common) for m in shard_maps]
    kw = {}
    if os.environ.get("KBENCH_TRACE"):
        kw = dict(trace=True, tmpdir=os.environ.get("KBENCH_TMPDIR") or None)
    res = run_bass_kernel_spmd(nc, in_maps, list(range(CORES)), **kw)
    t3 = time.time()
    LAST_INFO.update(preprocess_s=t1 - t0, build_s=t2 - t1, run_s=t3 - t2,
                     exec_time_ns=res.exec_time_ns)
    out = np.concatenate(
        [res.results[k]["out"][0, :cfg.S] for k in range(CORES)])
    return out.reshape(cfg.N, 1).astype(np.float32)


def kernel(**inputs):
    return _run(inputs, FULL)


# revision 32
# speedup vs baseline: 1.7740x; 1.1402x over previous
"""GCN encoder/decoder (gnn_message_passing) Trainium2 kernel.

Pull-model with PE segment-sum aggregation:
  - nodes partitioned across 8 cores (owner-computes on dst)
  - per conv: AllGather fp16 feature table; tokens (edges incl self loops)
    sorted by dst 128-block and grouped by table quarter (so dma_gather
    indices fit int16); per 128-token chunk, gather src rows and
    segment-sum them on the PE: psum[feat, seg] += msg[tok, feat]^T @
    S[tok, seg] with S built on-chip (iota==segid), accumulating per
    dst-block in PSUM.  No scatter-add, no HBM round trip for y.
  - dinv[dst] applied during PSUM eviction (broadcast multiply), weight
    GEMM from fp16 staging, BN stats fused into eviction (accum_out),
    1KB AllReduce, scalar-engine affine+ReLU, table rebuild (PE
    transpose + dinv[src] scale + fp16 cast).
"""

import math
import os
import time
from contextlib import ExitStack

import numpy as np

CORES = 8
H = 128
EPS = 1e-5


class Cfg:
    def __init__(self, N, depth=9, sblk=8, cap=8, queues=1, scratch=16384,
                 fused_sbuild=False):
        assert N % CORES == 0
        self.N = N
        self.S = N // CORES
        self.NPC = ((self.S + 127) // 128) * 128
        self.NBLK = self.NPC // 128
        self.QROWS = 2 * self.NPC          # table rows per quarter (2 shards)
        assert self.QROWS <= 32767
        self.depth = depth
        self.nconv = 2 * depth + 1
        self.SB = min(sblk, self.NBLK)     # dst blocks per superblock
        self.NSUP = (self.NBLK + self.SB - 1) // self.SB
        self.CAP = cap                     # max chunks per gather call
        self.QUEUES = queues
        self.SCRATCH = scratch
        self.FUSED_SBUILD = fused_sbuild
        assert cap * 128 <= scratch // 16


FULL = Cfg(100000, fused_sbuild=True, queues=4)


# ----------------------------------------------------------------------------
# Host-side preprocessing (sharding / token planning)
# ----------------------------------------------------------------------------

def wrap16(a):
    # token i -> [i % 16, i // 16], replicated to 128 partitions
    b = a.astype(np.int16).reshape(-1, 16).T.copy()
    return np.tile(b, (8, 1))


def preprocess(x, edge_index, cfg):
    N, S, NPC, QROWS, NBLK, SB, CAP = (cfg.N, cfg.S, cfg.NPC, cfg.QROWS,
                                       cfg.NBLK, cfg.SB, cfg.CAP)
    # self loops are folded into eviction as dinv^2 * z_prev (no tokens)
    src = np.asarray(edge_index[0], np.int64)
    dst = np.asarray(edge_index[1], np.int64)
    deg = np.bincount(dst, minlength=N).astype(np.float32) + 1.0
    dinv = (1.0 / np.sqrt(deg)).astype(np.float32)

    gid = (src // S) * NPC + (src % S)     # padded global row id in table
    shard = dst // S
    NPAIR = (NBLK + 1) // 2                # segment window = 2 blocks (256)
    NG = 4 * NPAIR                         # (quarter, pair) groups

    per_core = []
    cnt = np.zeros((CORES, NG), np.int64)
    for k in range(CORES):
        m = shard == k
        g = gid[m]
        d = dst[m] - k * S
        q = g // QROWS
        lrow = g % QROWS
        bp = d // 256
        key = q * NPAIR + bp
        order = np.argsort(key, kind="stable")
        per_core.append((lrow[order], (d % 256)[order]))
        bounds = np.searchsorted(key[order], np.arange(NG + 1))
        per_core[k] = per_core[k] + (bounds,)
        cnt[k] = np.diff(bounds)

    nch = ((cnt + 127) // 128).max(axis=0)             # [NG] static plan
    for bp in range(NPAIR):                # every pair >=1 chunk
        if nch[bp::NPAIR].sum() == 0:
            nch[bp] = 1

    # PSUM accumulation groups are per 2KB bank = 2 pairs (4 dst blocks);
    # start/stop flags must be one per bank (start marks it pending-zero).
    total_per_pair = nch.reshape(4, NPAIR).sum(axis=0)
    NBANK = (NPAIR + 1) // 2
    bank_total = np.zeros(NBANK, np.int64)
    for bp in range(NPAIR):
        bank_total[bp // 2] += total_per_pair[bp]
    bank_seen = np.zeros(NBANK, np.int64)
    SBP = SB // 2                          # pairs per superblock
    plan = []          # [sb] -> list of calls
    segcol = 0
    off16 = 0
    for sbi in range(cfg.NSUP):
        pairs = list(range(sbi * SBP, min((sbi + 1) * SBP, NPAIR)))
        sb_calls = []
        for q in range(4):
            chunk_ids = [(bp, i) for bp in pairs
                         for i in range(int(nch[q * NPAIR + bp]))]
            pos = 0
            while pos < len(chunk_ids):
                take = chunk_ids[pos:pos + CAP]
                descs = []
                for jslot, (bp, i) in enumerate(take):
                    bk = bp // 2
                    first = bank_seen[bk] == 0
                    bank_seen[bk] += 1
                    last = bank_seen[bk] == bank_total[bk]
                    descs.append((jslot, bp - sbi * SBP, segcol, bool(first),
                                  bool(last)))
                    segcol += 1
                sb_calls.append(dict(q=q, chunks=take, n=len(take),
                                     off16=off16, descs=descs))
                off16 += len(take) * 8
                pos += len(take)
        plan.append(sb_calls)
    nchk = segcol

    in_maps = []
    for k in range(CORES):
        lrow_k, seg_k, bounds = per_core[k]
        idx_cols, seg_cols = [], []
        for sb_calls in plan:
            for call in sb_calls:
                q = call["q"]
                L, Sg = [], []
                for (bp, i) in call["chunks"]:
                    gq = q * NPAIR + bp
                    lo, hi = int(bounds[gq]), int(bounds[gq + 1])
                    s0 = lo + i * 128
                    rows = np.zeros(128, np.int64)
                    segs = np.full(128, -1, np.int64)
                    n = max(0, min(hi - s0, 128))
                    if n > 0:
                        rows[:n] = lrow_k[s0:s0 + n]
                        segs[:n] = seg_k[s0:s0 + n]
                    L.append(rows)
                    Sg.append(segs)
                idx_cols.append(wrap16(np.concatenate(L)))
                seg_cols.append(np.stack(Sg))
        IDX = np.concatenate(idx_cols, axis=1)
        SEGID = np.ascontiguousarray(
            np.concatenate(seg_cols, axis=0).T.astype(np.float32))

        xt = np.zeros((x.shape[1], NPC), dtype=np.float32)
        xt[:, :S] = np.asarray(x[k * S:(k + 1) * S], np.float32).T
        dv = np.zeros(NPC, dtype=np.float32)
        dv[:S] = dinv[k * S:(k + 1) * S]
        dinv_nm = np.ascontiguousarray(dv.reshape(NBLK, 128).T)
        dinvb = np.ascontiguousarray(
            np.broadcast_to(dv, (128, NPC)).astype(np.float16))
        dinv2b = np.ascontiguousarray(
            np.broadcast_to(dv * dv, (128, NPC)).astype(np.float16))
        in_maps.append({"xT": xt, "gidx": IDX, "segid": SEGID,
                        "dinv_nm": dinv_nm, "dinvb": dinvb,
                        "dinv2b": dinv2b})
    return in_maps, plan, nchk


# ----------------------------------------------------------------------------
# Device kernel
# ----------------------------------------------------------------------------

def build_nc(cfg, plan, nchk, d_in):
    import concourse.bacc as bacc
    import concourse.bass as bass
    import concourse.mybir as mybir
    import concourse.tile as tile

    f32 = mybir.dt.float32
    f16 = mybir.dt.float16
    i16 = mybir.dt.int16
    AF = mybir.ActivationFunctionType
    ALU = mybir.AluOpType
    AX = mybir.AxisListType

    NPC, NBLK, SB, QROWS = cfg.NPC, cfg.NBLK, cfg.SB, cfg.QROWS
    depth = cfg.depth
    nconv = cfg.nconv
    TROWS = CORES * NPC
    MAXSLOT = max(c["n"] for sb in plan for c in sb)
    NS = (NPC + 511) // 512
    STATC = 2 * cfg.NSUP + 2

    # descriptor carveout: ring must hold a whole gather call (CAP*128 descs)
    nc = bacc.Bacc("TRN2", target_bir_lowering=False, debug=False,
                   num_devices=CORES,
                   dynamic_dma_scratch_size=cfg.SCRATCH,
                   num_swdge_queues=cfg.QUEUES)

    # ---- I/O ----
    xT_d = nc.dram_tensor("xT", [d_in, NPC], f32, kind="ExternalInput")
    gidx_d = nc.dram_tensor("gidx", [128, nchk * 8], i16, kind="ExternalInput")
    segid_d = nc.dram_tensor("segid", [128, nchk], f32, kind="ExternalInput")
    dinvnm_d = nc.dram_tensor("dinv_nm", [128, NBLK], f32, kind="ExternalInput")
    dinvb_d = nc.dram_tensor("dinvb", [128, NPC], f16, kind="ExternalInput")
    dinv2b_d = nc.dram_tensor("dinv2b", [128, NPC], f16, kind="ExternalInput")
    W0_d = nc.dram_tensor("W0", [d_in, H], f32, kind="ExternalInput")
    Ws1_d = nc.dram_tensor("Ws1", [depth, H, H], f16, kind="ExternalInput")
    Ws2_d = nc.dram_tensor("Ws2", [depth - 1, H, H], f16, kind="ExternalInput")
    Wout_d = nc.dram_tensor("Wout", [H, 1], f16, kind="ExternalInput")
    g1_d = nc.dram_tensor("g1T", [H, depth + 1], f32, kind="ExternalInput")
    b1_d = nc.dram_tensor("b1T", [H, depth + 1], f32, kind="ExternalInput")
    g2_d = nc.dram_tensor("g2T", [H, depth - 1], f32, kind="ExternalInput")
    b2_d = nc.dram_tensor("b2T", [H, depth - 1], f32, kind="ExternalInput")
    ident_d = nc.dram_tensor("ident", [128, 128], f32, kind="ExternalInput")
    out_d = nc.dram_tensor("out", [1, NPC], f32, kind="ExternalOutput")

    # ---- internals ----
    tabs = [nc.dram_tensor(f"tab{i}", [TROWS, H], f16, addr_space="Shared")
            for i in range(2)]
    ulocal = nc.dram_tensor("ulocal", [NPC, H], f16)
    stats_in = nc.dram_tensor("stats_in", [128, 2], f32)
    stats_out = nc.dram_tensor("stats_out", [128, 2], f32, addr_space="Shared")
    xs_d = nc.dram_tensor("xs", [depth, 128, NPC], f32)

    rg = [list(range(CORES))]

    with tile.TileContext(nc, num_cores=CORES) as tc, ExitStack() as ctx:
        persist = ctx.enter_context(tc.tile_pool(name="persist", bufs=1))
        msgp = ctx.enter_context(tc.tile_pool(name="msg", bufs=3))
        sp = ctx.enter_context(tc.tile_pool(name="sbld", bufs=2))
        ytp = ctx.enter_context(tc.tile_pool(name="yt", bufs=2))
        stgp = ctx.enter_context(tc.tile_pool(name="stg", bufs=3))
        wp = ctx.enter_context(tc.tile_pool(name="wp", bufs=2))
        skp = ctx.enter_context(tc.tile_pool(name="skp", bufs=3))
        smallp = ctx.enter_context(tc.tile_pool(name="small", bufs=8))
        obp = ctx.enter_context(tc.tile_pool(name="obp", bufs=2))
        accp = ctx.enter_context(tc.tile_pool(name="accp", bufs=2, space="PSUM"))
        pgemm = ctx.enter_context(tc.tile_pool(name="pgemm", bufs=2, space="PSUM"))

        # persistent tiles
        zbuf = persist.tile([128, NPC], f32)
        idx_sb = persist.tile([128, nchk * 8], i16)
        segid_sb = persist.tile([128, nchk], f32)
        dinvb_sb = persist.tile([128, NPC], f16)
        dinv2b_sb = persist.tile([128, NPC], f16)
        dinvnm_sb = persist.tile([128, NBLK], f32)
        iota_sb = persist.tile([128, 256], f32)
        ident_sb = persist.tile([128, 128], f32)
        sums_sb = persist.tile([128, STATC], f32)
        sumsq_sb = persist.tile([128, STATC], f32)
        stat2_sb = persist.tile([128, 2], f32)
        sqscr = persist.tile([128, 512], f32)
        wout_sb = persist.tile([128, 1], f16)

        # load persistent data (split large loads across DMA queues)
        PIECE = 8192 * 2  # int16 elems per partition-row piece
        tot16 = nchk * 8
        o = 0
        while o < tot16:
            w = min(PIECE, tot16 - o)
            nc.sync.dma_start(out=idx_sb[:, o:o + w], in_=gidx_d[:, o:o + w])
            o += w
        nc.sync.dma_start(out=segid_sb[:], in_=segid_d[:])
        o = 0
        while o < NPC:
            w = min(4096, NPC - o)
            nc.sync.dma_start(out=dinvb_sb[:, o:o + w], in_=dinvb_d[:, o:o + w])
            nc.sync.dma_start(out=dinv2b_sb[:, o:o + w],
                              in_=dinv2b_d[:, o:o + w])
            o += w
        nc.sync.dma_start(out=dinvnm_sb[:], in_=dinvnm_d[:])
        nc.sync.dma_start(out=ident_sb[:], in_=ident_d[:])
        nc.sync.dma_start(out=wout_sb[:], in_=Wout_d[:])
        nc.gpsimd.iota(iota_sb[:], pattern=[[1, 256]], base=0,
                       channel_multiplier=0,
                       allow_small_or_imprecise_dtypes=True)

        def gemm_weight(t):
            if t == 1 or t == nconv:
                return None
            w = wp.tile([128, 128], f16, tag="w")
            if t <= depth + 1:
                nc.sync.dma_start(out=w[:], in_=Ws1_d[t - 2])
            else:
                nc.sync.dma_start(out=w[:], in_=Ws2_d[t - depth - 2])
            return w

        def bn_params(t):
            gt = smallp.tile([128, 1], f32, tag="gt")
            bt = smallp.tile([128, 1], f32, tag="bt")
            if t <= depth + 1:
                nc.sync.dma_start(out=gt[:], in_=g1_d[:, t - 1:t])
                nc.sync.dma_start(out=bt[:], in_=b1_d[:, t - 1:t])
            else:
                i = t - depth - 2
                nc.sync.dma_start(out=gt[:], in_=g2_d[:, i:i + 1])
                nc.sync.dma_start(out=bt[:], in_=b2_d[:, i:i + 1])
            return gt, bt

        def build_table(t):
            # zbuf (feature-major fp32) -> transpose -> dinv[src] -> fp16
            NB4 = (NBLK + 3) // 4
            for g in range(NB4):
                b0 = 4 * g
                nb = min(4, NBLK - b0)
                st = stgp.tile([128, 4, H], f16, tag="st")
                pt = pgemm.tile([128, 512], f32, tag="pg", name="pt")
                for j in range(nb):
                    b = b0 + j
                    nc.tensor.transpose(
                        pt[:, j * 128:(j + 1) * 128],
                        zbuf[:, b * 128:(b + 1) * 128], ident_sb[:])
                    nc.vector.tensor_scalar_mul(
                        st[:, j, :], pt[:, j * 128:(j + 1) * 128],
                        dinvnm_sb[:, b:b + 1])
                nc.sync.dma_start(
                    out=ulocal[b0 * 128:(b0 + nb) * 128, :]
                    .rearrange("(a p) f -> p a f", p=128),
                    in_=st[:, :nb, :])
            nc.gpsimd.collective_compute(
                "AllGather", ALU.bypass, replica_groups=rg,
                ins=[ulocal[:, :]], outs=[tabs[t % 2][:, :]])

        # ---- stage 0: z0.T = W0.T @ xT ----
        w0 = persist.tile([d_in, H], f32)
        nc.sync.dma_start(out=w0[:], in_=W0_d[:])
        for s in range(NS):
            c0 = s * 512
            cw = min(512, NPC - c0)
            xt = skp.tile([d_in, 512], f32, tag="xt")
            nc.sync.dma_start(out=xt[:, :cw], in_=xT_d[:, c0:c0 + cw])
            pg = pgemm.tile([128, 512], f32, tag="pg")
            nc.tensor.matmul(pg[:, :cw], w0[:], xt[:, :cw],
                             start=True, stop=True)
            nc.scalar.copy(zbuf[:, c0:c0 + cw], pg[:, :cw])
        build_table(0)

        # ---- conv layers ----
        qrr = 0
        for t in range(1, nconv + 1):
            tab = tabs[(t - 1) % 2]
            w = gemm_weight(t)
            scol = 0
            for sbi in range(cfg.NSUP):
                nb_sb = min(SB, NBLK - sbi * SB)
                nacct = (nb_sb + 3) // 4
                acct = [accp.tile([128, 512], f32, tag=f"acct{i}",
                                  name=f"acct{i}")
                        for i in range(nacct)]
                accs = [acct[j // 4][:, (j % 4) * 128:(j % 4 + 1) * 128]
                        for j in range(nb_sb)]
                for call in plan[sbi]:
                    q, ncall, off16 = call["q"], call["n"], call["off16"]
                    msg = msgp.tile([128, MAXSLOT, H], f16, tag="msg")
                    nc.gpsimd.dma_gather(
                        msg[:, :ncall, :],
                        tab[q * QROWS:(q + 1) * QROWS, :],
                        idx_sb[:, off16:off16 + ncall * 8],
                        ncall * 128, ncall * 128, H,
                        queue_num=qrr % cfg.QUEUES)
                    qrr += 1
                    c0 = call["descs"][0][2]
                    st_ = sp.tile([128, MAXSLOT, 256], f16, tag="S")
                    if cfg.FUSED_SBUILD:
                        nc.vector.tensor_tensor(
                            st_[:, :ncall, :],
                            iota_sb[:].unsqueeze(1)
                            .broadcast_to([128, ncall, 256]),
                            segid_sb[:, c0:c0 + ncall].unsqueeze(2)
                            .broadcast_to([128, ncall, 256]),
                            op=ALU.is_equal)
                    else:
                        for jj in range(ncall):
                            nc.vector.tensor_scalar(
                                st_[:, jj, :], iota_sb[:],
                                segid_sb[:, c0 + jj:c0 + jj + 1], None,
                                op0=ALU.is_equal)
                    for (jslot, jp, segc, first, last) in call["descs"]:
                        nc.tensor.matmul(
                            acct[jp // 2][:, (jp % 2) * 256:
                                          (jp % 2) * 256 + 256],
                            msg[:, jslot, :], st_[:, jslot, :],
                            start=first, stop=last)

                # ---- evict superblock (y*dinv + self loop dinv^2*z_prev) ----
                nb0 = sbi * SB * 128
                if t == 1:
                    for j in range(nb_sb):
                        cols = slice(nb0 + j * 128, nb0 + (j + 1) * 128)
                        stmp = ytp.tile([128, 128], f16, tag="slf", name="stmp")
                        nc.vector.tensor_mul(stmp[:], zbuf[:, cols],
                                             dinv2b_sb[:, cols])
                        nc.vector.tensor_mul(zbuf[:, cols], accs[j],
                                             dinvb_sb[:, cols])
                        nc.vector.tensor_add(zbuf[:, cols], zbuf[:, cols],
                                             stmp[:])
                    continue
                ytmp = ytp.tile([128, SB * 128], f16, tag="ytmp")
                for j in range(nb_sb):
                    cols = slice(nb0 + j * 128, nb0 + (j + 1) * 128)
                    ycols = slice(j * 128, (j + 1) * 128)
                    stmp = ytp.tile([128, 128], f16, tag="slf", name="stmp")
                    nc.vector.tensor_mul(stmp[:], zbuf[:, cols],
                                         dinv2b_sb[:, cols])
                    nc.vector.tensor_mul(ytmp[:, ycols], accs[j],
                                         dinvb_sb[:, cols])
                    nc.vector.tensor_add(ytmp[:, ycols], ytmp[:, ycols],
                                         stmp[:])
                for hw_ in range(0, nb_sb * 128, 512):
                    cw = min(512, nb_sb * 128 - hw_)
                    if t < nconv:
                        pg = pgemm.tile([128, 512], f32, tag="pg")
                        nc.tensor.matmul(pg[:, :cw], w[:], ytmp[:, hw_:hw_ + cw],
                                         start=True, stop=True)
                        nc.scalar.activation(
                            zbuf[:, nb0 + hw_: nb0 + hw_ + cw], pg[:, :cw],
                            AF.Copy, accum_out=sums_sb[:, scol:scol + 1])
                        nc.scalar.activation(
                            sqscr[:, :cw], pg[:, :cw],
                            AF.Square, accum_out=sumsq_sb[:, scol:scol + 1])
                        scol += 1
                    else:
                        po = pgemm.tile([128, 512], f32, tag="pg", name="po")
                        nc.tensor.matmul(po[0:1, :cw], wout_sb[:],
                                         ytmp[:, hw_:hw_ + cw],
                                         start=True, stop=True)
                        ob = obp.tile([1, 512], f32, tag="ob")
                        nc.scalar.activation(ob[:, :cw], po[0:1, :cw],
                                             AF.Sigmoid)
                        nc.sync.dma_start(
                            out=out_d[:, nb0 + hw_: nb0 + hw_ + cw],
                            in_=ob[:, :cw])

            if t == nconv:
                break

            # ---- BN stats ----
            if t == 1:
                for s in range(NS):
                    c0 = s * 512
                    cw = min(512, NPC - c0)
                    zsl = zbuf[:, c0:c0 + cw]
                    nc.vector.tensor_reduce(sums_sb[:, s:s + 1], zsl,
                                            axis=AX.X, op=ALU.add)
                    nc.vector.tensor_mul(sqscr[:, :cw], zsl, zsl)
                    nc.vector.tensor_reduce(sumsq_sb[:, s:s + 1], sqscr[:, :cw],
                                            axis=AX.X, op=ALU.add)
                scol = NS
            nc.vector.tensor_reduce(stat2_sb[:, 0:1], sums_sb[:, :scol],
                                    axis=AX.X, op=ALU.add)
            nc.vector.tensor_reduce(stat2_sb[:, 1:2], sumsq_sb[:, :scol],
                                    axis=AX.X, op=ALU.add)
            nc.sync.dma_start(out=stats_in[:, :], in_=stat2_sb[:])
            nc.gpsimd.collective_compute(
                "AllReduce", ALU.add, replica_groups=rg,
                ins=[stats_in[:, :]], outs=[stats_out[:, :]])
            gst = smallp.tile([128, 2], f32, tag="gst")
            nc.sync.dma_start(out=gst[:], in_=stats_out[:, :])

            mean = smallp.tile([128, 1], f32, tag="mean")
            m2 = smallp.tile([128, 1], f32, tag="m2")
            var = smallp.tile([128, 1], f32, tag="var")
            scl = smallp.tile([128, 1], f32, tag="scl")
            sft = smallp.tile([128, 1], f32, tag="sft")
            inv_n = 1.0 / float(cfg.N)
            nc.vector.tensor_scalar_mul(mean[:], gst[:, 0:1], inv_n)
            nc.vector.tensor_scalar_mul(var[:], gst[:, 1:2], inv_n)
            nc.vector.tensor_mul(m2[:], mean[:], mean[:])
            nc.vector.scalar_tensor_tensor(
                var[:], m2[:], -1.0, var[:], op0=ALU.mult, op1=ALU.add)
            nc.vector.tensor_scalar_add(var[:], var[:], EPS)
            gt, bt = bn_params(t)
            nc.scalar.sqrt(scl[:], var[:])
            nc.vector.reciprocal(scl[:], scl[:])
            nc.vector.tensor_mul(scl[:], scl[:], gt[:])
            nc.vector.tensor_mul(sft[:], mean[:], scl[:])
            nc.vector.scalar_tensor_tensor(
                sft[:], sft[:], -1.0, bt[:], op0=ALU.mult, op1=ALU.add)

            # ---- normalize + relu (in place on zbuf) ----
            for s in range(NS):
                c0 = s * 512
                cw = min(512, NPC - c0)
                nc.scalar.activation(zbuf[:, c0:c0 + cw], zbuf[:, c0:c0 + cw],
                                     AF.Relu, bias=sft[:], scale=scl[:])

            # ---- stash xs / skip add ----
            if t <= depth:
                for s in range(NS):
                    c0 = s * 512
                    cw = min(512, NPC - c0)
                    nc.sync.dma_start(out=xs_d[t - 1][:, c0:c0 + cw],
                                      in_=zbuf[:, c0:c0 + cw])
            if t + 1 >= depth + 2:
                j = 2 * depth - t
                for s in range(NS):
                    c0 = s * 512
                    cw = min(512, NPC - c0)
                    sk = skp.tile([128, 512], f32, tag="sk")
                    nc.sync.dma_start(out=sk[:, :cw], in_=xs_d[j][:, c0:c0 + cw])
                    nc.vector.tensor_add(zbuf[:, c0:c0 + cw],
                                         zbuf[:, c0:c0 + cw], sk[:, :cw])

            build_table(t)

    nc.compile()
    return nc


# ----------------------------------------------------------------------------
# Entry point
# ----------------------------------------------------------------------------

LAST_INFO = {}


def _run(inputs, cfg):
    from concourse.bass_utils import run_bass_kernel_spmd

    x = np.asarray(inputs["x"], np.float32)
    d_in = x.shape[1]
    t0 = time.time()
    shard_maps, plan, nchk = preprocess(x, inputs["edge_index"], cfg)
    t1 = time.time()
    nc = build_nc(cfg, plan, nchk, d_in)
    t2 = time.time()

    common = {
        "W0": np.asarray(inputs["W0"], np.float32),
        "Ws1": np.asarray(inputs["Ws1"], np.float16),
        "Ws2": np.asarray(inputs["Ws2"], np.float16),
        "Wout": np.asarray(inputs["Wout"], np.float16),
        "g1T": np.ascontiguousarray(np.asarray(inputs["g1"], np.float32).T),
        "b1T": np.ascontiguousarray(np.asarray(inputs["b1"], np.float32).T),
        "g2T": np.ascontiguousarray(np.asarray(inputs["g2"], np.float32).T),
        "b2T": np.ascontiguousarray(np.asarray(inputs["b2"], np.float32).T),
        "ident": np.eye(128, dtype=np.float32),
    }
    in_maps = [dict(m, **common) for m in shard_maps]
    kw = {}
    if os.environ.get("KBENCH_TRACE"):
        kw = dict(trace=True, tmpdir=os.environ.get("KBENCH_TMPDIR") or None)
    res = run_bass_kernel_spmd(nc, in_maps, list(range(CORES)), **kw)
    t3 = time.time()
    LAST_INFO.update(preprocess_s=t1 - t0, build_s=t2 - t1, run_s=t3 - t2,
                     exec_time_ns=res.exec_time_ns, nchk=nchk)
    out = np.concatenate(
        [res.results[k]["out"][0, :cfg.S] for k in range(CORES)])
    return out.reshape(cfg.N, 1).astype(np.float32)


def kernel(**inputs):
    return _run(inputs, FULL)


# revision 34
# speedup vs baseline: 2.1522x; 1.2132x over previous
"""GCN encoder/decoder (gnn_message_passing) Trainium2 kernel.

Pull-model with PE segment-sum aggregation:
  - nodes partitioned across 8 cores (owner-computes on dst)
  - per conv: AllGather fp16 feature table; tokens (edges incl self loops)
    sorted by dst 128-block and grouped by table quarter (so dma_gather
    indices fit int16); per 128-token chunk, gather src rows and
    segment-sum them on the PE: psum[feat, seg] += msg[tok, feat]^T @
    S[tok, seg] with S built on-chip (iota==segid), accumulating per
    dst-block in PSUM.  No scatter-add, no HBM round trip for y.
  - dinv[dst] applied during PSUM eviction (broadcast multiply), weight
    GEMM from fp16 staging, BN stats fused into eviction (accum_out),
    1KB AllReduce, scalar-engine affine+ReLU, table rebuild (PE
    transpose + dinv[src] scale + fp16 cast).
"""

import math
import os
import time
from contextlib import ExitStack

import numpy as np

CORES = 8
H = 128
EPS = 1e-5


class Cfg:
    def __init__(self, N, depth=9, sblk=8, cap=8, queues=1, scratch=16384,
                 fused_sbuild=False):
        assert N % CORES == 0
        self.N = N
        self.S = N // CORES
        self.NPC = ((self.S + 127) // 128) * 128
        self.NBLK = self.NPC // 128
        self.QROWS = 2 * self.NPC          # table rows per quarter (2 shards)
        assert self.QROWS <= 32767
        self.depth = depth
        self.nconv = 2 * depth + 1
        self.SB = min(sblk, self.NBLK)     # dst blocks per superblock
        self.NSUP = (self.NBLK + self.SB - 1) // self.SB
        self.CAP = cap                     # max chunks per gather call
        self.QUEUES = queues
        self.SCRATCH = scratch
        self.FUSED_SBUILD = fused_sbuild
        assert cap * 128 <= scratch // 16


FULL = Cfg(100000, fused_sbuild=True, queues=4)


# ----------------------------------------------------------------------------
# Host-side preprocessing (sharding / token planning)
# ----------------------------------------------------------------------------

def wrap16(a):
    # token i -> [i % 16, i // 16], replicated to 128 partitions
    b = a.astype(np.int16).reshape(-1, 16).T.copy()
    return np.tile(b, (8, 1))


def preprocess(x, edge_index, cfg):
    N, S, NPC, QROWS, NBLK, SB, CAP = (cfg.N, cfg.S, cfg.NPC, cfg.QROWS,
                                       cfg.NBLK, cfg.SB, cfg.CAP)
    # self loops are folded into eviction as dinv^2 * z_prev (no tokens)
    src = np.asarray(edge_index[0], np.int64)
    dst = np.asarray(edge_index[1], np.int64)
    deg = np.bincount(dst, minlength=N).astype(np.float32) + 1.0
    dinv = (1.0 / np.sqrt(deg)).astype(np.float32)

    gid = (src // S) * NPC + (src % S)     # padded global row id in table
    shard = dst // S
    NPAIR = (NBLK + 1) // 2                # segment window = 2 blocks (256)
    NG = 4 * NPAIR                         # (quarter, pair) groups

    per_core = []
    cnt = np.zeros((CORES, NG), np.int64)
    for k in range(CORES):
        m = shard == k
        g = gid[m]
        d = dst[m] - k * S
        q = g // QROWS
        lrow = g % QROWS
        bp = d // 256
        key = q * NPAIR + bp
        order = np.argsort(key, kind="stable")
        per_core.append((lrow[order], (d % 256)[order]))
        bounds = np.searchsorted(key[order], np.arange(NG + 1))
        per_core[k] = per_core[k] + (bounds,)
        cnt[k] = np.diff(bounds)

    nch = ((cnt + 127) // 128).max(axis=0)             # [NG] static plan
    for bp in range(NPAIR):                # every pair >=1 chunk
        if nch[bp::NPAIR].sum() == 0:
            nch[bp] = 1

    # PSUM accumulation groups are per 2KB bank = 2 pairs (4 dst blocks);
    # start/stop flags must be one per bank (start marks it pending-zero).
    total_per_pair = nch.reshape(4, NPAIR).sum(axis=0)
    NBANK = (NPAIR + 1) // 2
    bank_total = np.zeros(NBANK, np.int64)
    for bp in range(NPAIR):
        bank_total[bp // 2] += total_per_pair[bp]
    bank_seen = np.zeros(NBANK, np.int64)
    SBP = SB // 2                          # pairs per superblock
    plan = []          # [sb] -> list of calls
    segcol = 0
    off16 = 0
    for sbi in range(cfg.NSUP):
        pairs = list(range(sbi * SBP, min((sbi + 1) * SBP, NPAIR)))
        sb_calls = []
        for q in range(4):
            chunk_ids = [(bp, i) for bp in pairs
                         for i in range(int(nch[q * NPAIR + bp]))]
            pos = 0
            while pos < len(chunk_ids):
                take = chunk_ids[pos:pos + CAP]
                descs = []
                for jslot, (bp, i) in enumerate(take):
                    bk = bp // 2
                    first = bank_seen[bk] == 0
                    bank_seen[bk] += 1
                    last = bank_seen[bk] == bank_total[bk]
                    descs.append((jslot, bp - sbi * SBP, segcol, bool(first),
                                  bool(last)))
                    segcol += 1
                sb_calls.append(dict(q=q, chunks=take, n=len(take),
                                     off16=off16, descs=descs))
                off16 += len(take) * 8
                pos += len(take)
        plan.append(sb_calls)
    nchk = segcol

    in_maps = []
    for k in range(CORES):
        lrow_k, seg_k, bounds = per_core[k]
        idx_cols, seg_cols = [], []
        for sb_calls in plan:
            for call in sb_calls:
                q = call["q"]
                L, Sg = [], []
                for (bp, i) in call["chunks"]:
                    gq = q * NPAIR + bp
                    lo, hi = int(bounds[gq]), int(bounds[gq + 1])
                    s0 = lo + i * 128
                    rows = np.zeros(128, np.int64)
                    segs = np.full(128, -1, np.int64)
                    n = max(0, min(hi - s0, 128))
                    if n > 0:
                        rows[:n] = lrow_k[s0:s0 + n]
                        segs[:n] = seg_k[s0:s0 + n]
                    L.append(rows)
                    Sg.append(segs)
                idx_cols.append(wrap16(np.concatenate(L)))
                seg_cols.append(np.stack(Sg))
        IDX = np.concatenate(idx_cols, axis=1)
        SEGID = np.ascontiguousarray(
            np.concatenate(seg_cols, axis=0).T.astype(np.float16))

        xt = np.zeros((x.shape[1], NPC), dtype=np.float32)
        xt[:, :S] = np.asarray(x[k * S:(k + 1) * S], np.float32).T
        dv = np.zeros(NPC, dtype=np.float32)
        dv[:S] = dinv[k * S:(k + 1) * S]
        dinv_nm = np.ascontiguousarray(dv.reshape(NBLK, 128).T)
        dinvb = np.ascontiguousarray(
            np.broadcast_to(dv, (128, NPC)).astype(np.float16))
        dinv2b = np.ascontiguousarray(
            np.broadcast_to(dv * dv, (128, NPC)).astype(np.float16))
        in_maps.append({"xT": xt, "gidx": IDX, "segid": SEGID,
                        "dinv_nm": dinv_nm, "dinvb": dinvb,
                        "dinv2b": dinv2b})
    return in_maps, plan, nchk


# ----------------------------------------------------------------------------
# Device kernel
# ----------------------------------------------------------------------------

def build_nc(cfg, plan, nchk, d_in):
    import concourse.bacc as bacc
    import concourse.bass as bass
    import concourse.mybir as mybir
    import concourse.tile as tile

    f32 = mybir.dt.float32
    f16 = mybir.dt.float16
    i16 = mybir.dt.int16
    AF = mybir.ActivationFunctionType
    ALU = mybir.AluOpType
    AX = mybir.AxisListType

    NPC, NBLK, SB, QROWS = cfg.NPC, cfg.NBLK, cfg.SB, cfg.QROWS
    depth = cfg.depth
    nconv = cfg.nconv
    TROWS = CORES * NPC
    MAXSLOT = max(c["n"] for sb in plan for c in sb)
    NS = (NPC + 511) // 512
    STATC = 2 * cfg.NSUP + 2

    # descriptor carveout: ring must hold a whole gather call (CAP*128 descs)
    nc = bacc.Bacc("TRN2", target_bir_lowering=False, debug=False,
                   num_devices=CORES,
                   dynamic_dma_scratch_size=cfg.SCRATCH,
                   num_swdge_queues=cfg.QUEUES)

    # ---- I/O ----
    xT_d = nc.dram_tensor("xT", [d_in, NPC], f32, kind="ExternalInput")
    gidx_d = nc.dram_tensor("gidx", [128, nchk * 8], i16, kind="ExternalInput")
    segid_d = nc.dram_tensor("segid", [128, nchk], f16, kind="ExternalInput")
    dinvnm_d = nc.dram_tensor("dinv_nm", [128, NBLK], f32, kind="ExternalInput")
    dinvb_d = nc.dram_tensor("dinvb", [128, NPC], f16, kind="ExternalInput")
    dinv2b_d = nc.dram_tensor("dinv2b", [128, NPC], f16, kind="ExternalInput")
    W0_d = nc.dram_tensor("W0", [d_in, H], f32, kind="ExternalInput")
    Ws1_d = nc.dram_tensor("Ws1", [depth, H, H], f16, kind="ExternalInput")
    Ws2_d = nc.dram_tensor("Ws2", [depth - 1, H, H], f16, kind="ExternalInput")
    Wout_d = nc.dram_tensor("Wout", [H, 1], f16, kind="ExternalInput")
    g1_d = nc.dram_tensor("g1T", [H, depth + 1], f32, kind="ExternalInput")
    b1_d = nc.dram_tensor("b1T", [H, depth + 1], f32, kind="ExternalInput")
    g2_d = nc.dram_tensor("g2T", [H, depth - 1], f32, kind="ExternalInput")
    b2_d = nc.dram_tensor("b2T", [H, depth - 1], f32, kind="ExternalInput")
    ident_d = nc.dram_tensor("ident", [128, 128], f32, kind="ExternalInput")
    out_d = nc.dram_tensor("out", [1, NPC], f32, kind="ExternalOutput")

    # ---- internals ----
    tabs = [nc.dram_tensor(f"tab{i}", [TROWS, H], f16, addr_space="Shared")
            for i in range(2)]
    ulocal = nc.dram_tensor("ulocal", [NPC, H], f16)
    stats_in = nc.dram_tensor("stats_in", [128, 2], f32)
    stats_out = nc.dram_tensor("stats_out", [128, 2], f32, addr_space="Shared")
    xs_d = nc.dram_tensor("xs", [depth, 128, NPC], f32)

    rg = [list(range(CORES))]

    with tile.TileContext(nc, num_cores=CORES) as tc, ExitStack() as ctx:
        persist = ctx.enter_context(tc.tile_pool(name="persist", bufs=1))
        msgp = ctx.enter_context(tc.tile_pool(name="msg", bufs=4))
        sp = ctx.enter_context(tc.tile_pool(name="sbld", bufs=3))
        ytp = ctx.enter_context(tc.tile_pool(name="yt", bufs=2))
        stgp = ctx.enter_context(tc.tile_pool(name="stg", bufs=3))
        wp = ctx.enter_context(tc.tile_pool(name="wp", bufs=2))
        skp = ctx.enter_context(tc.tile_pool(name="skp", bufs=3))
        smallp = ctx.enter_context(tc.tile_pool(name="small", bufs=8))
        obp = ctx.enter_context(tc.tile_pool(name="obp", bufs=2))
        accp = ctx.enter_context(tc.tile_pool(name="accp", bufs=3, space="PSUM"))
        pgemm = ctx.enter_context(tc.tile_pool(name="pgemm", bufs=2, space="PSUM"))

        # persistent tiles
        zbuf = persist.tile([128, NPC], f32)
        idx_sb = persist.tile([128, nchk * 8], i16)
        segid_sb = persist.tile([128, nchk], f16)
        dinvb_sb = persist.tile([128, NPC], f16)
        dinv2b_sb = persist.tile([128, NPC], f16)
        dinvnm_sb = persist.tile([128, NBLK], f32)
        iota_sb = persist.tile([128, 256], f16)
        ident_sb = persist.tile([128, 128], f32)
        sums_sb = persist.tile([128, STATC], f32)
        sumsq_sb = persist.tile([128, STATC], f32)
        stat2_sb = persist.tile([128, 2], f32)
        sqscr = persist.tile([128, 512], f32)
        wout_sb = persist.tile([128, 1], f16)

        # load persistent data (split large loads across DMA queues)
        PIECE = 8192 * 2  # int16 elems per partition-row piece
        tot16 = nchk * 8
        o = 0
        while o < tot16:
            w = min(PIECE, tot16 - o)
            nc.sync.dma_start(out=idx_sb[:, o:o + w], in_=gidx_d[:, o:o + w])
            o += w
        nc.sync.dma_start(out=segid_sb[:], in_=segid_d[:])
        o = 0
        while o < NPC:
            w = min(4096, NPC - o)
            nc.sync.dma_start(out=dinvb_sb[:, o:o + w], in_=dinvb_d[:, o:o + w])
            nc.sync.dma_start(out=dinv2b_sb[:, o:o + w],
                              in_=dinv2b_d[:, o:o + w])
            o += w
        nc.sync.dma_start(out=dinvnm_sb[:], in_=dinvnm_d[:])
        nc.sync.dma_start(out=ident_sb[:], in_=ident_d[:])
        nc.sync.dma_start(out=wout_sb[:], in_=Wout_d[:])
        nc.gpsimd.iota(iota_sb[:], pattern=[[1, 256]], base=0,
                       channel_multiplier=0,
                       allow_small_or_imprecise_dtypes=True)

        def gemm_weight(t):
            if t == 1 or t == nconv:
                return None
            w = wp.tile([128, 128], f16, tag="w")
            if t <= depth + 1:
                nc.sync.dma_start(out=w[:], in_=Ws1_d[t - 2])
            else:
                nc.sync.dma_start(out=w[:], in_=Ws2_d[t - depth - 2])
            return w

        def bn_params(t):
            gt = smallp.tile([128, 1], f32, tag="gt")
            bt = smallp.tile([128, 1], f32, tag="bt")
            if t <= depth + 1:
                nc.sync.dma_start(out=gt[:], in_=g1_d[:, t - 1:t])
                nc.sync.dma_start(out=bt[:], in_=b1_d[:, t - 1:t])
            else:
                i = t - depth - 2
                nc.sync.dma_start(out=gt[:], in_=g2_d[:, i:i + 1])
                nc.sync.dma_start(out=bt[:], in_=b2_d[:, i:i + 1])
            return gt, bt

        def build_table(t):
            # zbuf (feature-major fp32) -> transpose -> dinv[src] -> fp16
            NB4 = (NBLK + 3) // 4
            for g in range(NB4):
                b0 = 4 * g
                nb = min(4, NBLK - b0)
                st = stgp.tile([128, 4, H], f16, tag="st")
                pt = pgemm.tile([128, 512], f32, tag="pg", name="pt")
                for j in range(nb):
                    b = b0 + j
                    nc.tensor.transpose(
                        pt[:, j * 128:(j + 1) * 128],
                        zbuf[:, b * 128:(b + 1) * 128], ident_sb[:])
                    nc.vector.tensor_scalar_mul(
                        st[:, j, :], pt[:, j * 128:(j + 1) * 128],
                        dinvnm_sb[:, b:b + 1])
                nc.sync.dma_start(
                    out=ulocal[b0 * 128:(b0 + nb) * 128, :]
                    .rearrange("(a p) f -> p a f", p=128),
                    in_=st[:, :nb, :])
            nc.gpsimd.collective_compute(
                "AllGather", ALU.bypass, replica_groups=rg,
                ins=[ulocal[:, :]], outs=[tabs[t % 2][:, :]])

        # ---- stage 0: z0.T = W0.T @ xT ----
        w0 = persist.tile([d_in, H], f32)
        nc.sync.dma_start(out=w0[:], in_=W0_d[:])
        for s in range(NS):
            c0 = s * 512
            cw = min(512, NPC - c0)
            xt = skp.tile([d_in, 512], f32, tag="xt")
            nc.sync.dma_start(out=xt[:, :cw], in_=xT_d[:, c0:c0 + cw])
            pg = pgemm.tile([128, 512], f32, tag="pg")
            nc.tensor.matmul(pg[:, :cw], w0[:], xt[:, :cw],
                             start=True, stop=True)
            nc.scalar.copy(zbuf[:, c0:c0 + cw], pg[:, :cw])
        build_table(0)

        # ---- conv layers ----
        qrr = 0
        for t in range(1, nconv + 1):
            tab = tabs[(t - 1) % 2]
            w = gemm_weight(t)
            scol = 0
            for sbi in range(cfg.NSUP):
                nb_sb = min(SB, NBLK - sbi * SB)
                nacct = (nb_sb + 3) // 4
                acct = [accp.tile([128, 512], f32, tag=f"acct{i}",
                                  name=f"acct{i}")
                        for i in range(nacct)]
                accs = [acct[j // 4][:, (j % 4) * 128:(j % 4 + 1) * 128]
                        for j in range(nb_sb)]
                for call in plan[sbi]:
                    q, ncall, off16 = call["q"], call["n"], call["off16"]
                    msg = msgp.tile([128, MAXSLOT, H], f16, tag="msg")
                    nc.gpsimd.dma_gather(
                        msg[:, :ncall, :],
                        tab[q * QROWS:(q + 1) * QROWS, :],
                        idx_sb[:, off16:off16 + ncall * 8],
                        ncall * 128, ncall * 128, H,
                        queue_num=qrr % cfg.QUEUES)
                    qrr += 1
                    c0 = call["descs"][0][2]
                    st_ = sp.tile([128, MAXSLOT, 256], f16, tag="S")
                    nc.vector.tensor_tensor(
                        st_[:, :ncall, :],
                        iota_sb[:].unsqueeze(1)
                        .broadcast_to([128, ncall, 256]),
                        segid_sb[:, c0:c0 + ncall].unsqueeze(2)
                        .broadcast_to([128, ncall, 256]),
                        op=ALU.is_equal)
                    for (jslot, jp, segc, first, last) in call["descs"]:
                        nc.tensor.matmul(
                            acct[jp // 2][:, (jp % 2) * 256:
                                          (jp % 2) * 256 + 256],
                            msg[:, jslot, :], st_[:, jslot, :],
                            start=first, stop=last)

                # ---- evict superblock (y*dinv + self loop dinv^2*z_prev) ----
                nb0 = sbi * SB * 128
                if t == 1:
                    for j in range(nb_sb):
                        cols = slice(nb0 + j * 128, nb0 + (j + 1) * 128)
                        stmp = ytp.tile([128, 128], f16, tag="slf", name="stmp")
                        nc.vector.tensor_mul(stmp[:], zbuf[:, cols],
                                             dinv2b_sb[:, cols])
                        nc.vector.tensor_mul(zbuf[:, cols], accs[j],
                                             dinvb_sb[:, cols])
                        nc.vector.tensor_add(zbuf[:, cols], zbuf[:, cols],
                                             stmp[:])
                    continue
                ytmp = ytp.tile([128, SB * 128], f16, tag="ytmp")
                for j in range(nb_sb):
                    cols = slice(nb0 + j * 128, nb0 + (j + 1) * 128)
                    ycols = slice(j * 128, (j + 1) * 128)
                    stmp = ytp.tile([128, 128], f16, tag="slf", name="stmp")
                    nc.vector.tensor_mul(stmp[:], zbuf[:, cols],
                                         dinv2b_sb[:, cols])
                    nc.vector.tensor_mul(ytmp[:, ycols], accs[j],
                                         dinvb_sb[:, cols])
                    nc.vector.tensor_add(ytmp[:, ycols], ytmp[:, ycols],
                                         stmp[:])
                for hw_ in range(0, nb_sb * 128, 512):
                    cw = min(512, nb_sb * 128 - hw_)
                    if t < nconv:
                        pg = pgemm.tile([128, 512], f32, tag="pg")
                        nc.tensor.matmul(pg[:, :cw], w[:], ytmp[:, hw_:hw_ + cw],
                                         start=True, stop=True)
                        nc.scalar.activation(
                            zbuf[:, nb0 + hw_: nb0 + hw_ + cw], pg[:, :cw],
                            AF.Copy, accum_out=sums_sb[:, scol:scol + 1])
                        nc.scalar.activation(
                            sqscr[:, :cw], pg[:, :cw],
                            AF.Square, accum_out=sumsq_sb[:, scol:scol + 1])
                        scol += 1
                    else:
                        po = pgemm.tile([128, 512], f32, tag="pg", name="po")
                        nc.tensor.matmul(po[0:1, :cw], wout_sb[:],
                                         ytmp[:, hw_:hw_ + cw],
                                         start=True, stop=True)
                        ob = obp.tile([1, 512], f32, tag="ob")
                        nc.scalar.activation(ob[:, :cw], po[0:1, :cw],
                                             AF.Sigmoid)
                        nc.sync.dma_start(
                            out=out_d[:, nb0 + hw_: nb0 + hw_ + cw],
                            in_=ob[:, :cw])

            if t == nconv:
                break

            # ---- BN stats ----
            if t == 1:
                for s in range(NS):
                    c0 = s * 512
                    cw = min(512, NPC - c0)
                    zsl = zbuf[:, c0:c0 + cw]
                    nc.vector.tensor_reduce(sums_sb[:, s:s + 1], zsl,
                                            axis=AX.X, op=ALU.add)
                    nc.vector.tensor_mul(sqscr[:, :cw], zsl, zsl)
                    nc.vector.tensor_reduce(sumsq_sb[:, s:s + 1], sqscr[:, :cw],
                                            axis=AX.X, op=ALU.add)
                scol = NS
            nc.vector.tensor_reduce(stat2_sb[:, 0:1], sums_sb[:, :scol],
                                    axis=AX.X, op=ALU.add)
            nc.vector.tensor_reduce(stat2_sb[:, 1:2], sumsq_sb[:, :scol],
                                    axis=AX.X, op=ALU.add)
            nc.sync.dma_start(out=stats_in[:, :], in_=stat2_sb[:])
            nc.gpsimd.collective_compute(
                "AllReduce", ALU.add, replica_groups=rg,
                ins=[stats_in[:, :]], outs=[stats_out[:, :]])
            gst = smallp.tile([128, 2], f32, tag="gst")
            nc.sync.dma_start(out=gst[:], in_=stats_out[:, :])

            mean = smallp.tile([128, 1], f32, tag="mean")
            m2 = smallp.tile([128, 1], f32, tag="m2")
            var = smallp.tile([128, 1], f32, tag="var")
            scl = smallp.tile([128, 1], f32, tag="scl")
            sft = smallp.tile([128, 1], f32, tag="sft")
            inv_n = 1.0 / float(cfg.N)
            nc.vector.tensor_scalar_mul(mean[:], gst[:, 0:1], inv_n)
            nc.vector.tensor_scalar_mul(var[:], gst[:, 1:2], inv_n)
            nc.vector.tensor_mul(m2[:], mean[:], mean[:])
            nc.vector.scalar_tensor_tensor(
                var[:], m2[:], -1.0, var[:], op0=ALU.mult, op1=ALU.add)
            nc.vector.tensor_scalar_add(var[:], var[:], EPS)
            gt, bt = bn_params(t)
            nc.scalar.sqrt(scl[:], var[:])
            nc.vector.reciprocal(scl[:], scl[:])
            nc.vector.tensor_mul(scl[:], scl[:], gt[:])
            nc.vector.tensor_mul(sft[:], mean[:], scl[:])
            nc.vector.scalar_tensor_tensor(
                sft[:], sft[:], -1.0, bt[:], op0=ALU.mult, op1=ALU.add)

            # ---- normalize + relu (in place on zbuf) ----
            for s in range(NS):
                c0 = s * 512
                cw = min(512, NPC - c0)
                nc.scalar.activation(zbuf[:, c0:c0 + cw], zbuf[:, c0:c0 + cw],
                                     AF.Relu, bias=sft[:], scale=scl[:])

            # ---- stash xs / skip add ----
            if t <= depth:
                for s in range(NS):
                    c0 = s * 512
                    cw = min(512, NPC - c0)
                    nc.sync.dma_start(out=xs_d[t - 1][:, c0:c0 + cw],
                                      in_=zbuf[:, c0:c0 + cw])
            if t + 1 >= depth + 2:
                j = 2 * depth - t
                for s in range(NS):
                    c0 = s * 512
                    cw = min(512, NPC - c0)
                    sk = skp.tile([128, 512], f32, tag="sk")
                    nc.sync.dma_start(out=sk[:, :cw], in_=xs_d[j][:, c0:c0 + cw])
                    nc.vector.tensor_add(zbuf[:, c0:c0 + cw],
                                         zbuf[:, c0:c0 + cw], sk[:, :cw])

            build_table(t)

    nc.compile()
    return nc


# ----------------------------------------------------------------------------
# Entry point
# ----------------------------------------------------------------------------

LAST_INFO = {}


def _run(inputs, cfg):
    from concourse.bass_utils import run_bass_kernel_spmd

    x = np.asarray(inputs["x"], np.float32)
    d_in = x.shape[1]
    t0 = time.time()
    shard_maps, plan, nchk = preprocess(x, inputs["edge_index"], cfg)
    t1 = time.time()
    nc = build_nc(cfg, plan, nchk, d_in)
    t2 = time.time()

    common = {
        "W0": np.asarray(inputs["W0"], np.float32),
        "Ws1": np.asarray(inputs["Ws1"], np.float16),
        "Ws2": np.asarray(inputs["Ws2"], np.float16),
        "Wout": np.asarray(inputs["Wout"], np.float16),
        "g1T": np.ascontiguousarray(np.asarray(inputs["g1"], np.float32).T),
        "b1T": np.ascontiguousarray(np.asarray(inputs["b1"], np.float32).T),
        "g2T": np.ascontiguousarray(np.asarray(inputs["g2"], np.float32).T),
        "b2T": np.ascontiguousarray(np.asarray(inputs["b2"], np.float32).T),
        "ident": np.eye(128, dtype=np.float32),
    }
    in_maps = [dict(m, **common) for m in shard_maps]
    kw = {}
    if os.environ.get("KBENCH_TRACE"):
        kw = dict(trace=True, tmpdir=os.environ.get("KBENCH_TMPDIR") or None)
    res = run_bass_kernel_spmd(nc, in_maps, list(range(CORES)), **kw)
    t3 = time.time()
    LAST_INFO.update(preprocess_s=t1 - t0, build_s=t2 - t1, run_s=t3 - t2,
                     exec_time_ns=res.exec_time_ns, nchk=nchk)
    out = np.concatenate(
        [res.results[k]["out"][0, :cfg.S] for k in range(CORES)])
    return out.reshape(cfg.N, 1).astype(np.float32)


def kernel(**inputs):
    return _run(inputs, FULL)


# revision 35
# speedup vs baseline: 2.2914x; 1.0647x over previous
"""GCN encoder/decoder (gnn_message_passing) Trainium2 kernel.

Pull-model with PE segment-sum aggregation:
  - nodes partitioned across 8 cores (owner-computes on dst)
  - per conv: AllGather fp16 feature table; tokens (edges incl self loops)
    sorted by dst 128-block and grouped by table quarter (so dma_gather
    indices fit int16); per 128-token chunk, gather src rows and
    segment-sum them on the PE: psum[feat, seg] += msg[tok, feat]^T @
    S[tok, seg] with S built on-chip (iota==segid), accumulating per
    dst-block in PSUM.  No scatter-add, no HBM round trip for y.
  - dinv[dst] applied during PSUM eviction (broadcast multiply), weight
    GEMM from fp16 staging, BN stats fused into eviction (accum_out),
    1KB AllReduce, scalar-engine affine+ReLU, table rebuild (PE
    transpose + dinv[src] scale + fp16 cast).
"""

import math
import os
import time
from contextlib import ExitStack

import numpy as np

CORES = 8
H = 128
EPS = 1e-5


class Cfg:
    def __init__(self, N, depth=9, sblk=8, cap=8, queues=1, scratch=16384,
                 fused_sbuild=False):
        assert N % CORES == 0
        self.N = N
        self.S = N // CORES
        self.NPC = ((self.S + 127) // 128) * 128
        self.NBLK = self.NPC // 128
        self.QROWS = 2 * self.NPC          # table rows per quarter (2 shards)
        assert self.QROWS <= 32767
        self.depth = depth
        self.nconv = 2 * depth + 1
        self.SB = min(sblk, self.NBLK)     # dst blocks per superblock
        self.NSUP = (self.NBLK + self.SB - 1) // self.SB
        self.CAP = cap                     # max chunks per gather call
        self.QUEUES = queues
        self.SCRATCH = scratch
        self.FUSED_SBUILD = fused_sbuild
        assert cap * 128 <= scratch // 16


FULL = Cfg(100000, fused_sbuild=True, queues=4)


# ----------------------------------------------------------------------------
# Host-side preprocessing (sharding / token planning)
# ----------------------------------------------------------------------------

def wrap16(a):
    # token i -> [i % 16, i // 16], replicated to 128 partitions
    b = a.astype(np.int16).reshape(-1, 16).T.copy()
    return np.tile(b, (8, 1))


def preprocess(x, edge_index, cfg):
    N, S, NPC, QROWS, NBLK, SB, CAP = (cfg.N, cfg.S, cfg.NPC, cfg.QROWS,
                                       cfg.NBLK, cfg.SB, cfg.CAP)
    # self loops are folded into eviction as dinv^2 * z_prev (no tokens)
    src = np.asarray(edge_index[0], np.int64)
    dst = np.asarray(edge_index[1], np.int64)
    deg = np.bincount(dst, minlength=N).astype(np.float32) + 1.0
    dinv = (1.0 / np.sqrt(deg)).astype(np.float32)

    gid = (src // S) * NPC + (src % S)     # padded global row id in table
    shard = dst // S
    NPAIR = (NBLK + 1) // 2                # segment window = 2 blocks (256)
    NG = 4 * NPAIR                         # (quarter, pair) groups

    per_core = []
    cnt = np.zeros((CORES, NG), np.int64)
    for k in range(CORES):
        m = shard == k
        g = gid[m]
        d = dst[m] - k * S
        q = g // QROWS
        lrow = g % QROWS
        bp = d // 256
        key = q * NPAIR + bp
        order = np.argsort(key, kind="stable")
        per_core.append((lrow[order], (d % 256)[order]))
        bounds = np.searchsorted(key[order], np.arange(NG + 1))
        per_core[k] = per_core[k] + (bounds,)
        cnt[k] = np.diff(bounds)

    nch = ((cnt + 127) // 128).max(axis=0)             # [NG] static plan
    for bp in range(NPAIR):                # every pair >=1 chunk
        if nch[bp::NPAIR].sum() == 0:
            nch[bp] = 1

    # PSUM accumulation groups are per 2KB bank = 2 pairs (4 dst blocks);
    # start/stop flags must be one per bank (start marks it pending-zero).
    total_per_pair = nch.reshape(4, NPAIR).sum(axis=0)
    NBANK = (NPAIR + 1) // 2
    bank_total = np.zeros(NBANK, np.int64)
    for bp in range(NPAIR):
        bank_total[bp // 2] += total_per_pair[bp]
    bank_seen = np.zeros(NBANK, np.int64)
    SBP = SB // 2                          # pairs per superblock
    plan = []          # [sb] -> list of calls
    segcol = 0
    off16 = 0
    for sbi in range(cfg.NSUP):
        pairs = list(range(sbi * SBP, min((sbi + 1) * SBP, NPAIR)))
        sb_calls = []
        for q in range(4):
            chunk_ids = [(bp, i) for bp in pairs
                         for i in range(int(nch[q * NPAIR + bp]))]
            pos = 0
            while pos < len(chunk_ids):
                take = chunk_ids[pos:pos + CAP]
                descs = []
                for jslot, (bp, i) in enumerate(take):
                    bk = bp // 2
                    first = bank_seen[bk] == 0
                    bank_seen[bk] += 1
                    last = bank_seen[bk] == bank_total[bk]
                    descs.append((jslot, bp - sbi * SBP, segcol, bool(first),
                                  bool(last)))
                    segcol += 1
                sb_calls.append(dict(q=q, chunks=take, n=len(take),
                                     off16=off16, descs=descs))
                off16 += len(take) * 8
                pos += len(take)
        plan.append(sb_calls)
    nchk = segcol

    in_maps = []
    for k in range(CORES):
        lrow_k, seg_k, bounds = per_core[k]
        idx_cols, seg_cols = [], []
        for sb_calls in plan:
            for call in sb_calls:
                q = call["q"]
                L, Sg = [], []
                for (bp, i) in call["chunks"]:
                    gq = q * NPAIR + bp
                    lo, hi = int(bounds[gq]), int(bounds[gq + 1])
                    s0 = lo + i * 128
                    rows = np.zeros(128, np.int64)
                    segs = np.full(128, -1, np.int64)
                    n = max(0, min(hi - s0, 128))
                    if n > 0:
                        rows[:n] = lrow_k[s0:s0 + n]
                        segs[:n] = seg_k[s0:s0 + n]
                    L.append(rows)
                    Sg.append(segs)
                idx_cols.append(wrap16(np.concatenate(L)))
                seg_cols.append(np.stack(Sg))
        IDX = np.concatenate(idx_cols, axis=1)
        SEGID = np.ascontiguousarray(
            np.concatenate(seg_cols, axis=0).T.astype(np.float16))

        xt = np.zeros((x.shape[1], NPC), dtype=np.float32)
        xt[:, :S] = np.asarray(x[k * S:(k + 1) * S], np.float32).T
        dv = np.zeros(NPC, dtype=np.float32)
        dv[:S] = dinv[k * S:(k + 1) * S]
        dinv_nm = np.ascontiguousarray(dv.reshape(NBLK, 128).T)
        dinvb = np.ascontiguousarray(
            np.broadcast_to(dv, (128, NPC)).astype(np.float16))
        dinv2b = np.ascontiguousarray(
            np.broadcast_to(dv * dv, (128, NPC)).astype(np.float16))
        in_maps.append({"xT": xt, "gidx": IDX, "segid": SEGID,
                        "dinv_nm": dinv_nm, "dinvb": dinvb,
                        "dinv2b": dinv2b})
    return in_maps, plan, nchk


# ----------------------------------------------------------------------------
# Device kernel
# ----------------------------------------------------------------------------

def build_nc(cfg, plan, nchk, d_in):
    import concourse.bacc as bacc
    import concourse.bass as bass
    import concourse.mybir as mybir
    import concourse.tile as tile

    f32 = mybir.dt.float32
    f16 = mybir.dt.float16
    i16 = mybir.dt.int16
    AF = mybir.ActivationFunctionType
    ALU = mybir.AluOpType
    AX = mybir.AxisListType

    NPC, NBLK, SB, QROWS = cfg.NPC, cfg.NBLK, cfg.SB, cfg.QROWS
    depth = cfg.depth
    nconv = cfg.nconv
    TROWS = CORES * NPC
    MAXSLOT = max(c["n"] for sb in plan for c in sb)
    NS = (NPC + 511) // 512
    STATC = 2 * cfg.NSUP + 2

    # descriptor carveout: ring must hold a whole gather call (CAP*128 descs)
    nc = bacc.Bacc("TRN2", target_bir_lowering=False, debug=False,
                   num_devices=CORES,
                   dynamic_dma_scratch_size=cfg.SCRATCH,
                   num_swdge_queues=cfg.QUEUES)

    # ---- I/O ----
    xT_d = nc.dram_tensor("xT", [d_in, NPC], f32, kind="ExternalInput")
    gidx_d = nc.dram_tensor("gidx", [128, nchk * 8], i16, kind="ExternalInput")
    segid_d = nc.dram_tensor("segid", [128, nchk], f16, kind="ExternalInput")
    dinvnm_d = nc.dram_tensor("dinv_nm", [128, NBLK], f32, kind="ExternalInput")
    dinvb_d = nc.dram_tensor("dinvb", [128, NPC], f16, kind="ExternalInput")
    dinv2b_d = nc.dram_tensor("dinv2b", [128, NPC], f16, kind="ExternalInput")
    W0_d = nc.dram_tensor("W0", [d_in, H], f32, kind="ExternalInput")
    Ws1_d = nc.dram_tensor("Ws1", [depth, H, H], f16, kind="ExternalInput")
    Ws2_d = nc.dram_tensor("Ws2", [depth - 1, H, H], f16, kind="ExternalInput")
    Wout_d = nc.dram_tensor("Wout", [H, 1], f16, kind="ExternalInput")
    g1_d = nc.dram_tensor("g1T", [H, depth + 1], f32, kind="ExternalInput")
    b1_d = nc.dram_tensor("b1T", [H, depth + 1], f32, kind="ExternalInput")
    g2_d = nc.dram_tensor("g2T", [H, depth - 1], f32, kind="ExternalInput")
    b2_d = nc.dram_tensor("b2T", [H, depth - 1], f32, kind="ExternalInput")
    ident_d = nc.dram_tensor("ident", [128, 128], f32, kind="ExternalInput")
    out_d = nc.dram_tensor("out", [1, NPC], f32, kind="ExternalOutput")

    # ---- internals ----
    tabs = [nc.dram_tensor(f"tab{i}", [TROWS, H], f16, addr_space="Shared")
            for i in range(2)]
    ulocal = nc.dram_tensor("ulocal", [NPC, H], f16)
    stats_in = nc.dram_tensor("stats_in", [128, 2], f32)
    stats_out = nc.dram_tensor("stats_out", [128, 2], f32, addr_space="Shared")
    xs_d = nc.dram_tensor("xs", [depth, 128, NPC], f32)

    rg = [list(range(CORES))]

    with tile.TileContext(nc, num_cores=CORES) as tc, ExitStack() as ctx:
        persist = ctx.enter_context(tc.tile_pool(name="persist", bufs=1))
        msgp = ctx.enter_context(tc.tile_pool(name="msg", bufs=6))
        sp = ctx.enter_context(tc.tile_pool(name="sbld", bufs=4))
        ytp = ctx.enter_context(tc.tile_pool(name="yt", bufs=2))
        stgp = ctx.enter_context(tc.tile_pool(name="stg", bufs=3))
        wp = ctx.enter_context(tc.tile_pool(name="wp", bufs=2))
        skp = ctx.enter_context(tc.tile_pool(name="skp", bufs=3))
        smallp = ctx.enter_context(tc.tile_pool(name="small", bufs=8))
        obp = ctx.enter_context(tc.tile_pool(name="obp", bufs=2))
        accp = ctx.enter_context(tc.tile_pool(name="accp", bufs=3, space="PSUM"))
        pgemm = ctx.enter_context(tc.tile_pool(name="pgemm", bufs=2, space="PSUM"))

        # persistent tiles
        zbuf = persist.tile([128, NPC], f32)
        idx_sb = persist.tile([128, nchk * 8], i16)
        segid_sb = persist.tile([128, nchk], f16)
        dinvb_sb = persist.tile([128, NPC], f16)
        dinv2b_sb = persist.tile([128, NPC], f16)
        dinvnm_sb = persist.tile([128, NBLK], f32)
        iota_sb = persist.tile([128, 256], f16)
        ident_sb = persist.tile([128, 128], f32)
        sums_sb = persist.tile([128, STATC], f32)
        sumsq_sb = persist.tile([128, STATC], f32)
        stat2_sb = persist.tile([128, 2], f32)
        sqscr = persist.tile([128, 512], f32)
        wout_sb = persist.tile([128, 1], f16)

        # load persistent data (split large loads across DMA queues)
        PIECE = 8192 * 2  # int16 elems per partition-row piece
        tot16 = nchk * 8
        o = 0
        while o < tot16:
            w = min(PIECE, tot16 - o)
            nc.sync.dma_start(out=idx_sb[:, o:o + w], in_=gidx_d[:, o:o + w])
            o += w
        nc.sync.dma_start(out=segid_sb[:], in_=segid_d[:])
        o = 0
        while o < NPC:
            w = min(4096, NPC - o)
            nc.sync.dma_start(out=dinvb_sb[:, o:o + w], in_=dinvb_d[:, o:o + w])
            nc.sync.dma_start(out=dinv2b_sb[:, o:o + w],
                              in_=dinv2b_d[:, o:o + w])
            o += w
        nc.sync.dma_start(out=dinvnm_sb[:], in_=dinvnm_d[:])
        nc.sync.dma_start(out=ident_sb[:], in_=ident_d[:])
        nc.sync.dma_start(out=wout_sb[:], in_=Wout_d[:])
        nc.gpsimd.iota(iota_sb[:], pattern=[[1, 256]], base=0,
                       channel_multiplier=0,
                       allow_small_or_imprecise_dtypes=True)

        def gemm_weight(t):
            if t == 1 or t == nconv:
                return None
            w = wp.tile([128, 128], f16, tag="w")
            if t <= depth + 1:
                nc.sync.dma_start(out=w[:], in_=Ws1_d[t - 2])
            else:
                nc.sync.dma_start(out=w[:], in_=Ws2_d[t - depth - 2])
            return w

        def bn_params(t):
            gt = smallp.tile([128, 1], f32, tag="gt")
            bt = smallp.tile([128, 1], f32, tag="bt")
            if t <= depth + 1:
                nc.sync.dma_start(out=gt[:], in_=g1_d[:, t - 1:t])
                nc.sync.dma_start(out=bt[:], in_=b1_d[:, t - 1:t])
            else:
                i = t - depth - 2
                nc.sync.dma_start(out=gt[:], in_=g2_d[:, i:i + 1])
                nc.sync.dma_start(out=bt[:], in_=b2_d[:, i:i + 1])
            return gt, bt

        def build_table(t):
            # zbuf (feature-major fp32) -> transpose -> dinv[src] -> fp16
            NB4 = (NBLK + 3) // 4
            for g in range(NB4):
                b0 = 4 * g
                nb = min(4, NBLK - b0)
                st = stgp.tile([128, 4, H], f16, tag="st")
                pt = pgemm.tile([128, 512], f32, tag="pg", name="pt")
                for j in range(nb):
                    b = b0 + j
                    nc.tensor.transpose(
                        pt[:, j * 128:(j + 1) * 128],
                        zbuf[:, b * 128:(b + 1) * 128], ident_sb[:])
                    nc.vector.tensor_scalar_mul(
                        st[:, j, :], pt[:, j * 128:(j + 1) * 128],
                        dinvnm_sb[:, b:b + 1])
                nc.sync.dma_start(
                    out=ulocal[b0 * 128:(b0 + nb) * 128, :]
                    .rearrange("(a p) f -> p a f", p=128),
                    in_=st[:, :nb, :])
            nc.gpsimd.collective_compute(
                "AllGather", ALU.bypass, replica_groups=rg,
                ins=[ulocal[:, :]], outs=[tabs[t % 2][:, :]])

        # ---- stage 0: z0.T = W0.T @ xT ----
        w0 = persist.tile([d_in, H], f32)
        nc.sync.dma_start(out=w0[:], in_=W0_d[:])
        for s in range(NS):
            c0 = s * 512
            cw = min(512, NPC - c0)
            xt = skp.tile([d_in, 512], f32, tag="xt")
            nc.sync.dma_start(out=xt[:, :cw], in_=xT_d[:, c0:c0 + cw])
            pg = pgemm.tile([128, 512], f32, tag="pg")
            nc.tensor.matmul(pg[:, :cw], w0[:], xt[:, :cw],
                             start=True, stop=True)
            nc.scalar.copy(zbuf[:, c0:c0 + cw], pg[:, :cw])
        build_table(0)

        # ---- conv layers ----
        qrr = 0
        for t in range(1, nconv + 1):
            tab = tabs[(t - 1) % 2]
            w = gemm_weight(t)
            scol = 0
            for sbi in range(cfg.NSUP):
                nb_sb = min(SB, NBLK - sbi * SB)
                nacct = (nb_sb + 3) // 4
                acct = [accp.tile([128, 512], f32, tag=f"acct{i}",
                                  name=f"acct{i}")
                        for i in range(nacct)]
                accs = [acct[j // 4][:, (j % 4) * 128:(j % 4 + 1) * 128]
                        for j in range(nb_sb)]
                for call in plan[sbi]:
                    q, ncall, off16 = call["q"], call["n"], call["off16"]
                    msg = msgp.tile([128, MAXSLOT, H], f16, tag="msg")
                    nc.gpsimd.dma_gather(
                        msg[:, :ncall, :],
                        tab[q * QROWS:(q + 1) * QROWS, :],
                        idx_sb[:, off16:off16 + ncall * 8],
                        ncall * 128, ncall * 128, H,
                        queue_num=qrr % cfg.QUEUES)
                    qrr += 1
                    c0 = call["descs"][0][2]
                    st_ = sp.tile([128, MAXSLOT, 256], f16, tag="S")
                    nc.vector.tensor_tensor(
                        st_[:, :ncall, :],
                        iota_sb[:].unsqueeze(1)
                        .broadcast_to([128, ncall, 256]),
                        segid_sb[:, c0:c0 + ncall].unsqueeze(2)
                        .broadcast_to([128, ncall, 256]),
                        op=ALU.is_equal)
                    for (jslot, jp, segc, first, last) in call["descs"]:
                        nc.tensor.matmul(
                            acct[jp // 2][:, (jp % 2) * 256:
                                          (jp % 2) * 256 + 256],
                            msg[:, jslot, :], st_[:, jslot, :],
                            start=first, stop=last)

                # ---- evict superblock (y*dinv + self loop dinv^2*z_prev) ----
                nb0 = sbi * SB * 128
                if t == 1:
                    for j in range(nb_sb):
                        cols = slice(nb0 + j * 128, nb0 + (j + 1) * 128)
                        stmp = ytp.tile([128, 128], f16, tag="slf", name="stmp")
                        nc.vector.tensor_mul(stmp[:], zbuf[:, cols],
                                             dinv2b_sb[:, cols])
                        nc.vector.tensor_mul(zbuf[:, cols], accs[j],
                                             dinvb_sb[:, cols])
                        nc.vector.tensor_add(zbuf[:, cols], zbuf[:, cols],
                                             stmp[:])
                    continue
                ytmp = ytp.tile([128, SB * 128], f16, tag="ytmp")
                for j in range(nb_sb):
                    cols = slice(nb0 + j * 128, nb0 + (j + 1) * 128)
                    ycols = slice(j * 128, (j + 1) * 128)
                    stmp = ytp.tile([128, 128], f16, tag="slf", name="stmp")
                    nc.vector.tensor_mul(stmp[:], zbuf[:, cols],
                                         dinv2b_sb[:, cols])
                    nc.vector.tensor_mul(ytmp[:, ycols], accs[j],
                                         dinvb_sb[:, cols])
                    nc.vector.tensor_add(ytmp[:, ycols], ytmp[:, ycols],
                                         stmp[:])
                for hw_ in range(0, nb_sb * 128, 512):
                    cw = min(512, nb_sb * 128 - hw_)
                    if t < nconv:
                        pg = pgemm.tile([128, 512], f32, tag="pg")
                        nc.tensor.matmul(pg[:, :cw], w[:], ytmp[:, hw_:hw_ + cw],
                                         start=True, stop=True)
                        nc.scalar.activation(
                            zbuf[:, nb0 + hw_: nb0 + hw_ + cw], pg[:, :cw],
                            AF.Copy, accum_out=sums_sb[:, scol:scol + 1])
                        nc.scalar.activation(
                            sqscr[:, :cw], pg[:, :cw],
                            AF.Square, accum_out=sumsq_sb[:, scol:scol + 1])
                        scol += 1
                    else:
                        po = pgemm.tile([128, 512], f32, tag="pg", name="po")
                        nc.tensor.matmul(po[0:1, :cw], wout_sb[:],
                                         ytmp[:, hw_:hw_ + cw],
                                         start=True, stop=True)
                        ob = obp.tile([1, 512], f32, tag="ob")
                        nc.scalar.activation(ob[:, :cw], po[0:1, :cw],
                                             AF.Sigmoid)
                        nc.sync.dma_start(
                            out=out_d[:, nb0 + hw_: nb0 + hw_ + cw],
                            in_=ob[:, :cw])

            if t == nconv:
                break

            # ---- BN stats ----
            if t == 1:
                for s in range(NS):
                    c0 = s * 512
                    cw = min(512, NPC - c0)
                    zsl = zbuf[:, c0:c0 + cw]
                    nc.vector.tensor_reduce(sums_sb[:, s:s + 1], zsl,
                                            axis=AX.X, op=ALU.add)
                    nc.vector.tensor_mul(sqscr[:, :cw], zsl, zsl)
                    nc.vector.tensor_reduce(sumsq_sb[:, s:s + 1], sqscr[:, :cw],
                                            axis=AX.X, op=ALU.add)
                scol = NS
            nc.vector.tensor_reduce(stat2_sb[:, 0:1], sums_sb[:, :scol],
                                    axis=AX.X, op=ALU.add)
            nc.vector.tensor_reduce(stat2_sb[:, 1:2], sumsq_sb[:, :scol],
                                    axis=AX.X, op=ALU.add)
            nc.sync.dma_start(out=stats_in[:, :], in_=stat2_sb[:])
            nc.gpsimd.collective_compute(
                "AllReduce", ALU.add, replica_groups=rg,
                ins=[stats_in[:, :]], outs=[stats_out[:, :]])
            gst = smallp.tile([128, 2], f32, tag="gst")
            nc.sync.dma_start(out=gst[:], in_=stats_out[:, :])

            mean = smallp.tile([128, 1], f32, tag="mean")
            m2 = smallp.tile([128, 1], f32, tag="m2")
            var = smallp.tile([128, 1], f32, tag="var")
            scl = smallp.tile([128, 1], f32, tag="scl")
            sft = smallp.tile([128, 1], f32, tag="sft")
            inv_n = 1.0 / float(cfg.N)
            nc.vector.tensor_scalar_mul(mean[:], gst[:, 0:1], inv_n)
            nc.vector.tensor_scalar_mul(var[:], gst[:, 1:2], inv_n)
            nc.vector.tensor_mul(m2[:], mean[:], mean[:])
            nc.vector.scalar_tensor_tensor(
                var[:], m2[:], -1.0, var[:], op0=ALU.mult, op1=ALU.add)
            nc.vector.tensor_scalar_add(var[:], var[:], EPS)
            gt, bt = bn_params(t)
            nc.scalar.sqrt(scl[:], var[:])
            nc.vector.reciprocal(scl[:], scl[:])
            nc.vector.tensor_mul(scl[:], scl[:], gt[:])
            nc.vector.tensor_mul(sft[:], mean[:], scl[:])
            nc.vector.scalar_tensor_tensor(
                sft[:], sft[:], -1.0, bt[:], op0=ALU.mult, op1=ALU.add)

            # ---- normalize + relu (in place on zbuf) ----
            for s in range(NS):
                c0 = s * 512
                cw = min(512, NPC - c0)
                nc.scalar.activation(zbuf[:, c0:c0 + cw], zbuf[:, c0:c0 + cw],
                                     AF.Relu, bias=sft[:], scale=scl[:])

            # ---- stash xs / skip add ----
            if t <= depth:
                for s in range(NS):
                    c0 = s * 512
                    cw = min(512, NPC - c0)
                    nc.sync.dma_start(out=xs_d[t - 1][:, c0:c0 + cw],
                                      in_=zbuf[:, c0:c0 + cw])
            if t + 1 >= depth + 2:
                j = 2 * depth - t
                for s in range(NS):
                    c0 = s * 512
                    cw = min(512, NPC - c0)
                    sk = skp.tile([128, 512], f32, tag="sk")
                    nc.sync.dma_start(out=sk[:, :cw], in_=xs_d[j][:, c0:c0 + cw])
                    nc.vector.tensor_add(zbuf[:, c0:c0 + cw],
                                         zbuf[:, c0:c0 + cw], sk[:, :cw])

            build_table(t)

    nc.compile()
    return nc


# ----------------------------------------------------------------------------
# Entry point
# ----------------------------------------------------------------------------

LAST_INFO = {}


def _run(inputs, cfg):
    from concourse.bass_utils import run_bass_kernel_spmd

    x = np.asarray(inputs["x"], np.float32)
    d_in = x.shape[1]
    t0 = time.time()
    shard_maps, plan, nchk = preprocess(x, inputs["edge_index"], cfg)
    t1 = time.time()
    nc = build_nc(cfg, plan, nchk, d_in)
    t2 = time.time()

    common = {
        "W0": np.asarray(inputs["W0"], np.float32),
        "Ws1": np.asarray(inputs["Ws1"], np.float16),
        "Ws2": np.asarray(inputs["Ws2"], np.float16),
        "Wout": np.asarray(inputs["Wout"], np.float16),
        "g1T": np.ascontiguousarray(np.asarray(inputs["g1"], np.float32).T),
        "b1T": np.ascontiguousarray(np.asarray(inputs["b1"], np.float32).T),
        "g2T": np.ascontiguousarray(np.asarray(inputs["g2"], np.float32).T),
        "b2T": np.ascontiguousarray(np.asarray(inputs["b2"], np.float32).T),
        "ident": np.eye(128, dtype=np.float32),
    }
    in_maps = [dict(m, **common) for m in shard_maps]
    kw = {}
    if os.environ.get("KBENCH_TRACE"):
        kw = dict(trace=True, tmpdir=os.environ.get("KBENCH_TMPDIR") or None)
    res = run_bass_kernel_spmd(nc, in_maps, list(range(CORES)), **kw)
    t3 = time.time()
    LAST_INFO.update(preprocess_s=t1 - t0, build_s=t2 - t1, run_s=t3 - t2,
                     exec_time_ns=res.exec_time_ns, nchk=nchk)
    out = np.concatenate(
        [res.results[k]["out"][0, :cfg.S] for k in range(CORES)])
    return out.reshape(cfg.N, 1).astype(np.float32)


def kernel(**inputs):
    return _run(inputs, FULL)


# revision 36
# speedup vs baseline: 2.6090x; 1.1386x over previous
"""GCN encoder/decoder (gnn_message_passing) Trainium2 kernel.

Pull-model with PE segment-sum aggregation:
  - nodes partitioned across 8 cores (owner-computes on dst)
  - per conv: AllGather fp16 feature table; tokens (edges incl self loops)
    sorted by dst 128-block and grouped by table quarter (so dma_gather
    indices fit int16); per 128-token chunk, gather src rows and
    segment-sum them on the PE: psum[feat, seg] += msg[tok, feat]^T @
    S[tok, seg] with S built on-chip (iota==segid), accumulating per
    dst-block in PSUM.  No scatter-add, no HBM round trip for y.
  - dinv[dst] applied during PSUM eviction (broadcast multiply), weight
    GEMM from fp16 staging, BN stats fused into eviction (accum_out),
    1KB AllReduce, scalar-engine affine+ReLU, table rebuild (PE
    transpose + dinv[src] scale + fp16 cast).
"""

import math
import os
import time
from contextlib import ExitStack

import numpy as np

CORES = 8
H = 128
EPS = 1e-5


class Cfg:
    def __init__(self, N, depth=9, sblk=8, cap=8, queues=1, scratch=16384,
                 fused_sbuild=False):
        assert N % CORES == 0
        self.N = N
        self.S = N // CORES
        self.NPC = ((self.S + 127) // 128) * 128
        self.NBLK = self.NPC // 128
        self.QROWS = 2 * self.NPC          # table rows per quarter (2 shards)
        assert self.QROWS <= 32767
        self.depth = depth
        self.nconv = 2 * depth + 1
        self.SB = min(sblk, self.NBLK)     # dst blocks per superblock
        self.NSUP = (self.NBLK + self.SB - 1) // self.SB
        self.CAP = cap                     # max chunks per gather call
        self.QUEUES = queues
        self.SCRATCH = scratch
        self.FUSED_SBUILD = fused_sbuild
        assert cap * 128 <= scratch // 16


FULL = Cfg(100000, fused_sbuild=True, queues=4)


# ----------------------------------------------------------------------------
# Host-side preprocessing (sharding / token planning)
# ----------------------------------------------------------------------------

def wrap16(a):
    # token i -> [i % 16, i // 16], replicated to 128 partitions
    b = a.astype(np.int16).reshape(-1, 16).T.copy()
    return np.tile(b, (8, 1))


def preprocess(x, edge_index, cfg):
    N, S, NPC, QROWS, NBLK, SB, CAP = (cfg.N, cfg.S, cfg.NPC, cfg.QROWS,
                                       cfg.NBLK, cfg.SB, cfg.CAP)
    # self loops are folded into eviction as dinv^2 * z_prev (no tokens)
    src = np.asarray(edge_index[0], np.int64)
    dst = np.asarray(edge_index[1], np.int64)
    deg = np.bincount(dst, minlength=N).astype(np.float32) + 1.0
    dinv = (1.0 / np.sqrt(deg)).astype(np.float32)

    gid = (src // S) * NPC + (src % S)     # padded global row id in table
    shard = dst // S
    NPAIR = (NBLK + 1) // 2                # segment window = 2 blocks (256)
    NG = 4 * NPAIR                         # (quarter, pair) groups

    per_core = []
    cnt = np.zeros((CORES, NG), np.int64)
    for k in range(CORES):
        m = shard == k
        g = gid[m]
        d = dst[m] - k * S
        q = g // QROWS
        lrow = g % QROWS
        bp = d // 256
        key = q * NPAIR + bp
        order = np.argsort(key, kind="stable")
        per_core.append((lrow[order], (d % 256)[order]))
        bounds = np.searchsorted(key[order], np.arange(NG + 1))
        per_core[k] = per_core[k] + (bounds,)
        cnt[k] = np.diff(bounds)

    nch = ((cnt + 127) // 128).max(axis=0)             # [NG] static plan
    for bp in range(NPAIR):                # every pair >=1 chunk
        if nch[bp::NPAIR].sum() == 0:
            nch[bp] = 1

    # PSUM accumulation groups are per 2KB bank = 2 pairs (4 dst blocks);
    # start/stop flags must be one per bank (start marks it pending-zero).
    total_per_pair = nch.reshape(4, NPAIR).sum(axis=0)
    NBANK = (NPAIR + 1) // 2
    bank_total = np.zeros(NBANK, np.int64)
    for bp in range(NPAIR):
        bank_total[bp // 2] += total_per_pair[bp]
    bank_seen = np.zeros(NBANK, np.int64)
    SBP = SB // 2                          # pairs per superblock
    plan = []          # [sb] -> list of calls
    segcol = 0
    off16 = 0
    for sbi in range(cfg.NSUP):
        pairs = list(range(sbi * SBP, min((sbi + 1) * SBP, NPAIR)))
        sb_calls = []
        for q in range(4):
            chunk_ids = [(bp, i) for bp in pairs
                         for i in range(int(nch[q * NPAIR + bp]))]
            pos = 0
            while pos < len(chunk_ids):
                take = chunk_ids[pos:pos + CAP]
                descs = []
                for jslot, (bp, i) in enumerate(take):
                    bk = bp // 2
                    first = bank_seen[bk] == 0
                    bank_seen[bk] += 1
                    last = bank_seen[bk] == bank_total[bk]
                    descs.append((jslot, bp - sbi * SBP, segcol, bool(first),
                                  bool(last)))
                    segcol += 1
                sb_calls.append(dict(q=q, chunks=take, n=len(take),
                                     off16=off16, descs=descs))
                off16 += len(take) * 8
                pos += len(take)
        plan.append(sb_calls)
    nchk = segcol

    in_maps = []
    for k in range(CORES):
        lrow_k, seg_k, bounds = per_core[k]
        idx_cols, seg_cols = [], []
        for sb_calls in plan:
            for call in sb_calls:
                q = call["q"]
                L, Sg = [], []
                for (bp, i) in call["chunks"]:
                    gq = q * NPAIR + bp
                    lo, hi = int(bounds[gq]), int(bounds[gq + 1])
                    s0 = lo + i * 128
                    rows = np.zeros(128, np.int64)
                    segs = np.full(128, -1, np.int64)
                    n = max(0, min(hi - s0, 128))
                    if n > 0:
                        rows[:n] = lrow_k[s0:s0 + n]
                        segs[:n] = seg_k[s0:s0 + n]
                    L.append(rows)
                    Sg.append(segs)
                idx_cols.append(wrap16(np.concatenate(L)))
                seg_cols.append(np.stack(Sg))
        IDX = np.concatenate(idx_cols, axis=1)
        SEGID = np.ascontiguousarray(
            np.concatenate(seg_cols, axis=0).T.astype(np.float16))

        xt = np.zeros((x.shape[1], NPC), dtype=np.float32)
        xt[:, :S] = np.asarray(x[k * S:(k + 1) * S], np.float32).T
        dv = np.zeros(NPC, dtype=np.float32)
        dv[:S] = dinv[k * S:(k + 1) * S]
        dinv_nm = np.ascontiguousarray(dv.reshape(NBLK, 128).T)
        dinvb = np.ascontiguousarray(
            np.broadcast_to(dv, (128, NPC)).astype(np.float16))
        dinv2b = np.ascontiguousarray(
            np.broadcast_to(dv * dv, (128, NPC)).astype(np.float16))
        in_maps.append({"xT": xt, "gidx": IDX, "segid": SEGID,
                        "dinv_nm": dinv_nm, "dinvb": dinvb,
                        "dinv2b": dinv2b})
    return in_maps, plan, nchk


# ----------------------------------------------------------------------------
# Device kernel
# ----------------------------------------------------------------------------

def build_nc(cfg, plan, nchk, d_in):
    import concourse.bacc as bacc
    import concourse.bass as bass
    import concourse.mybir as mybir
    import concourse.tile as tile

    f32 = mybir.dt.float32
    f16 = mybir.dt.float16
    i16 = mybir.dt.int16
    AF = mybir.ActivationFunctionType
    ALU = mybir.AluOpType
    AX = mybir.AxisListType

    NPC, NBLK, SB, QROWS = cfg.NPC, cfg.NBLK, cfg.SB, cfg.QROWS
    depth = cfg.depth
    nconv = cfg.nconv
    TROWS = CORES * NPC
    MAXSLOT = max(c["n"] for sb in plan for c in sb)
    NS = (NPC + 511) // 512
    STATC = 2 * cfg.NSUP + 2

    # descriptor carveout: ring must hold a whole gather call (CAP*128 descs)
    nc = bacc.Bacc("TRN2", target_bir_lowering=False, debug=False,
                   num_devices=CORES,
                   dynamic_dma_scratch_size=cfg.SCRATCH,
                   num_swdge_queues=cfg.QUEUES)

    # ---- I/O ----
    xT_d = nc.dram_tensor("xT", [d_in, NPC], f32, kind="ExternalInput")
    gidx_d = nc.dram_tensor("gidx", [128, nchk * 8], i16, kind="ExternalInput")
    segid_d = nc.dram_tensor("segid", [128, nchk], f16, kind="ExternalInput")
    dinvnm_d = nc.dram_tensor("dinv_nm", [128, NBLK], f32, kind="ExternalInput")
    dinvb_d = nc.dram_tensor("dinvb", [128, NPC], f16, kind="ExternalInput")
    dinv2b_d = nc.dram_tensor("dinv2b", [128, NPC], f16, kind="ExternalInput")
    W0_d = nc.dram_tensor("W0", [d_in, H], f32, kind="ExternalInput")
    Ws1_d = nc.dram_tensor("Ws1", [depth, H, H], f16, kind="ExternalInput")
    Ws2_d = nc.dram_tensor("Ws2", [depth - 1, H, H], f16, kind="ExternalInput")
    Wout_d = nc.dram_tensor("Wout", [H, 1], f16, kind="ExternalInput")
    g1_d = nc.dram_tensor("g1T", [H, depth + 1], f32, kind="ExternalInput")
    b1_d = nc.dram_tensor("b1T", [H, depth + 1], f32, kind="ExternalInput")
    g2_d = nc.dram_tensor("g2T", [H, depth - 1], f32, kind="ExternalInput")
    b2_d = nc.dram_tensor("b2T", [H, depth - 1], f32, kind="ExternalInput")
    ident_d = nc.dram_tensor("ident", [128, 128], f32, kind="ExternalInput")
    out_d = nc.dram_tensor("out", [1, NPC], f32, kind="ExternalOutput")

    # ---- internals ----
    tabs = [nc.dram_tensor(f"tab{i}", [TROWS, H], f16, addr_space="Shared")
            for i in range(2)]
    ulocal = nc.dram_tensor("ulocal", [NPC, H], f16)
    stats_in = nc.dram_tensor("stats_in", [128, 2], f32)
    stats_out = nc.dram_tensor("stats_out", [128, 2], f32, addr_space="Shared")
    xs_d = nc.dram_tensor("xs", [depth, 128, NPC], f32)

    rg = [list(range(CORES))]

    with tile.TileContext(nc, num_cores=CORES) as tc, ExitStack() as ctx:
        persist = ctx.enter_context(tc.tile_pool(name="persist", bufs=1))
        msgp = ctx.enter_context(tc.tile_pool(name="msg", bufs=10))
        sp = ctx.enter_context(tc.tile_pool(name="sbld", bufs=6))
        ytp = ctx.enter_context(tc.tile_pool(name="yt", bufs=2))
        stgp = ctx.enter_context(tc.tile_pool(name="stg", bufs=3))
        wp = ctx.enter_context(tc.tile_pool(name="wp", bufs=2))
        skp = ctx.enter_context(tc.tile_pool(name="skp", bufs=3))
        smallp = ctx.enter_context(tc.tile_pool(name="small", bufs=8))
        obp = ctx.enter_context(tc.tile_pool(name="obp", bufs=2))
        accp = ctx.enter_context(tc.tile_pool(name="accp", bufs=3, space="PSUM"))
        pgemm = ctx.enter_context(tc.tile_pool(name="pgemm", bufs=2, space="PSUM"))

        # persistent tiles
        zbuf = persist.tile([128, NPC], f32)
        idx_sb = persist.tile([128, nchk * 8], i16)
        segid_sb = persist.tile([128, nchk], f16)
        dinvb_sb = persist.tile([128, NPC], f16)
        dinv2b_sb = persist.tile([128, NPC], f16)
        dinvnm_sb = persist.tile([128, NBLK], f32)
        iota_sb = persist.tile([128, 256], f16)
        ident_sb = persist.tile([128, 128], f32)
        sums_sb = persist.tile([128, STATC], f32)
        sumsq_sb = persist.tile([128, STATC], f32)
        stat2_sb = persist.tile([128, 2], f32)
        sqscr = persist.tile([128, 512], f32)
        wout_sb = persist.tile([128, 1], f16)

        # load persistent data (split large loads across DMA queues)
        PIECE = 8192 * 2  # int16 elems per partition-row piece
        tot16 = nchk * 8
        o = 0
        while o < tot16:
            w = min(PIECE, tot16 - o)
            nc.sync.dma_start(out=idx_sb[:, o:o + w], in_=gidx_d[:, o:o + w])
            o += w
        nc.sync.dma_start(out=segid_sb[:], in_=segid_d[:])
        o = 0
        while o < NPC:
            w = min(4096, NPC - o)
            nc.sync.dma_start(out=dinvb_sb[:, o:o + w], in_=dinvb_d[:, o:o + w])
            nc.sync.dma_start(out=dinv2b_sb[:, o:o + w],
                              in_=dinv2b_d[:, o:o + w])
            o += w
        nc.sync.dma_start(out=dinvnm_sb[:], in_=dinvnm_d[:])
        nc.sync.dma_start(out=ident_sb[:], in_=ident_d[:])
        nc.sync.dma_start(out=wout_sb[:], in_=Wout_d[:])
        nc.gpsimd.iota(iota_sb[:], pattern=[[1, 256]], base=0,
                       channel_multiplier=0,
                       allow_small_or_imprecise_dtypes=True)

        def gemm_weight(t):
            if t == 1 or t == nconv:
                return None
            w = wp.tile([128, 128], f16, tag="w")
            if t <= depth + 1:
                nc.sync.dma_start(out=w[:], in_=Ws1_d[t - 2])
            else:
                nc.sync.dma_start(out=w[:], in_=Ws2_d[t - depth - 2])
            return w

        def bn_params(t):
            gt = smallp.tile([128, 1], f32, tag="gt")
            bt = smallp.tile([128, 1], f32, tag="bt")
            if t <= depth + 1:
                nc.sync.dma_start(out=gt[:], in_=g1_d[:, t - 1:t])
                nc.sync.dma_start(out=bt[:], in_=b1_d[:, t - 1:t])
            else:
                i = t - depth - 2
                nc.sync.dma_start(out=gt[:], in_=g2_d[:, i:i + 1])
                nc.sync.dma_start(out=bt[:], in_=b2_d[:, i:i + 1])
            return gt, bt

        def build_table(t):
            # zbuf (feature-major fp32) -> transpose -> dinv[src] -> fp16
            NB4 = (NBLK + 3) // 4
            for g in range(NB4):
                b0 = 4 * g
                nb = min(4, NBLK - b0)
                st = stgp.tile([128, 4, H], f16, tag="st")
                pt = pgemm.tile([128, 512], f32, tag="pg", name="pt")
                for j in range(nb):
                    b = b0 + j
                    nc.tensor.transpose(
                        pt[:, j * 128:(j + 1) * 128],
                        zbuf[:, b * 128:(b + 1) * 128], ident_sb[:])
                    nc.vector.tensor_scalar_mul(
                        st[:, j, :], pt[:, j * 128:(j + 1) * 128],
                        dinvnm_sb[:, b:b + 1])
                nc.sync.dma_start(
                    out=ulocal[b0 * 128:(b0 + nb) * 128, :]
                    .rearrange("(a p) f -> p a f", p=128),
                    in_=st[:, :nb, :])
            nc.gpsimd.collective_compute(
                "AllGather", ALU.bypass, replica_groups=rg,
                ins=[ulocal[:, :]], outs=[tabs[t % 2][:, :]])

        # ---- stage 0: z0.T = W0.T @ xT ----
        w0 = persist.tile([d_in, H], f32)
        nc.sync.dma_start(out=w0[:], in_=W0_d[:])
        for s in range(NS):
            c0 = s * 512
            cw = min(512, NPC - c0)
            xt = skp.tile([d_in, 512], f32, tag="xt")
            nc.sync.dma_start(out=xt[:, :cw], in_=xT_d[:, c0:c0 + cw])
            pg = pgemm.tile([128, 512], f32, tag="pg")
            nc.tensor.matmul(pg[:, :cw], w0[:], xt[:, :cw],
                             start=True, stop=True)
            nc.scalar.copy(zbuf[:, c0:c0 + cw], pg[:, :cw])
        build_table(0)

        # ---- conv layers ----
        qrr = 0
        for t in range(1, nconv + 1):
            tab = tabs[(t - 1) % 2]
            w = gemm_weight(t)
            scol = 0
            for sbi in range(cfg.NSUP):
                nb_sb = min(SB, NBLK - sbi * SB)
                nacct = (nb_sb + 3) // 4
                acct = [accp.tile([128, 512], f32, tag=f"acct{i}",
                                  name=f"acct{i}")
                        for i in range(nacct)]
                accs = [acct[j // 4][:, (j % 4) * 128:(j % 4 + 1) * 128]
                        for j in range(nb_sb)]
                for call in plan[sbi]:
                    q, ncall, off16 = call["q"], call["n"], call["off16"]
                    msg = msgp.tile([128, MAXSLOT, H], f16, tag="msg")
                    nc.gpsimd.dma_gather(
                        msg[:, :ncall, :],
                        tab[q * QROWS:(q + 1) * QROWS, :],
                        idx_sb[:, off16:off16 + ncall * 8],
                        ncall * 128, ncall * 128, H,
                        queue_num=qrr % cfg.QUEUES)
                    qrr += 1
                    c0 = call["descs"][0][2]
                    st_ = sp.tile([128, MAXSLOT, 256], f16, tag="S")
                    nc.vector.tensor_tensor(
                        st_[:, :ncall, :],
                        iota_sb[:].unsqueeze(1)
                        .broadcast_to([128, ncall, 256]),
                        segid_sb[:, c0:c0 + ncall].unsqueeze(2)
                        .broadcast_to([128, ncall, 256]),
                        op=ALU.is_equal)
                    for (jslot, jp, segc, first, last) in call["descs"]:
                        nc.tensor.matmul(
                            acct[jp // 2][:, (jp % 2) * 256:
                                          (jp % 2) * 256 + 256],
                            msg[:, jslot, :], st_[:, jslot, :],
                            start=first, stop=last)

                # ---- evict superblock (y*dinv + self loop dinv^2*z_prev) ----
                nb0 = sbi * SB * 128
                if t == 1:
                    for j in range(nb_sb):
                        cols = slice(nb0 + j * 128, nb0 + (j + 1) * 128)
                        stmp = ytp.tile([128, 128], f16, tag="slf", name="stmp")
                        nc.vector.tensor_mul(stmp[:], zbuf[:, cols],
                                             dinv2b_sb[:, cols])
                        nc.vector.tensor_mul(zbuf[:, cols], accs[j],
                                             dinvb_sb[:, cols])
                        nc.vector.tensor_add(zbuf[:, cols], zbuf[:, cols],
                                             stmp[:])
                    continue
                ytmp = ytp.tile([128, SB * 128], f16, tag="ytmp")
                for j in range(nb_sb):
                    cols = slice(nb0 + j * 128, nb0 + (j + 1) * 128)
                    ycols = slice(j * 128, (j + 1) * 128)
                    stmp = ytp.tile([128, 128], f16, tag="slf", name="stmp")
                    nc.vector.tensor_mul(stmp[:], zbuf[:, cols],
                                         dinv2b_sb[:, cols])
                    nc.vector.tensor_mul(ytmp[:, ycols], accs[j],
                                         dinvb_sb[:, cols])
                    nc.vector.tensor_add(ytmp[:, ycols], ytmp[:, ycols],
                                         stmp[:])
                for hw_ in range(0, nb_sb * 128, 512):
                    cw = min(512, nb_sb * 128 - hw_)
                    if t < nconv:
                        pg = pgemm.tile([128, 512], f32, tag="pg")
                        nc.tensor.matmul(pg[:, :cw], w[:], ytmp[:, hw_:hw_ + cw],
                                         start=True, stop=True)
                        nc.scalar.activation(
                            zbuf[:, nb0 + hw_: nb0 + hw_ + cw], pg[:, :cw],
                            AF.Copy, accum_out=sums_sb[:, scol:scol + 1])
                        nc.scalar.activation(
                            sqscr[:, :cw], pg[:, :cw],
                            AF.Square, accum_out=sumsq_sb[:, scol:scol + 1])
                        scol += 1
                    else:
                        po = pgemm.tile([128, 512], f32, tag="pg", name="po")
                        nc.tensor.matmul(po[0:1, :cw], wout_sb[:],
                                         ytmp[:, hw_:hw_ + cw],
                                         start=True, stop=True)
                        ob = obp.tile([1, 512], f32, tag="ob")
                        nc.scalar.activation(ob[:, :cw], po[0:1, :cw],
                                             AF.Sigmoid)
                        nc.sync.dma_start(
                            out=out_d[:, nb0 + hw_: nb0 + hw_ + cw],
                            in_=ob[:, :cw])

            if t == nconv:
                break

            # ---- BN stats ----
            if t == 1:
                for s in range(NS):
                    c0 = s * 512
                    cw = min(512, NPC - c0)
                    zsl = zbuf[:, c0:c0 + cw]
                    nc.vector.tensor_reduce(sums_sb[:, s:s + 1], zsl,
                                            axis=AX.X, op=ALU.add)
                    nc.vector.tensor_mul(sqscr[:, :cw], zsl, zsl)
                    nc.vector.tensor_reduce(sumsq_sb[:, s:s + 1], sqscr[:, :cw],
                                            axis=AX.X, op=ALU.add)
                scol = NS
            nc.vector.tensor_reduce(stat2_sb[:, 0:1], sums_sb[:, :scol],
                                    axis=AX.X, op=ALU.add)
            nc.vector.tensor_reduce(stat2_sb[:, 1:2], sumsq_sb[:, :scol],
                                    axis=AX.X, op=ALU.add)
            nc.sync.dma_start(out=stats_in[:, :], in_=stat2_sb[:])
            nc.gpsimd.collective_compute(
                "AllReduce", ALU.add, replica_groups=rg,
                ins=[stats_in[:, :]], outs=[stats_out[:, :]])
            gst = smallp.tile([128, 2], f32, tag="gst")
            nc.sync.dma_start(out=gst[:], in_=stats_out[:, :])

            mean = smallp.tile([128, 1], f32, tag="mean")
            m2 = smallp.tile([128, 1], f32, tag="m2")
            var = smallp.tile([128, 1], f32, tag="var")
            scl = smallp.tile([128, 1], f32, tag="scl")
            sft = smallp.tile([128, 1], f32, tag="sft")
            inv_n = 1.0 / float(cfg.N)
            nc.vector.tensor_scalar_mul(mean[:], gst[:, 0:1], inv_n)
            nc.vector.tensor_scalar_mul(var[:], gst[:, 1:2], inv_n)
            nc.vector.tensor_mul(m2[:], mean[:], mean[:])
            nc.vector.scalar_tensor_tensor(
                var[:], m2[:], -1.0, var[:], op0=ALU.mult, op1=ALU.add)
            nc.vector.tensor_scalar_add(var[:], var[:], EPS)
            gt, bt = bn_params(t)
            nc.scalar.sqrt(scl[:], var[:])
            nc.vector.reciprocal(scl[:], scl[:])
            nc.vector.tensor_mul(scl[:], scl[:], gt[:])
            nc.vector.tensor_mul(sft[:], mean[:], scl[:])
            nc.vector.scalar_tensor_tensor(
                sft[:], sft[:], -1.0, bt[:], op0=ALU.mult, op1=ALU.add)

            # ---- normalize + relu (in place on zbuf) ----
            for s in range(NS):
                c0 = s * 512
                cw = min(512, NPC - c0)
                nc.scalar.activation(zbuf[:, c0:c0 + cw], zbuf[:, c0:c0 + cw],
                                     AF.Relu, bias=sft[:], scale=scl[:])

            # ---- stash xs / skip add ----
            if t <= depth:
                for s in range(NS):
                    c0 = s * 512
                    cw = min(512, NPC - c0)
                    nc.sync.dma_start(out=xs_d[t - 1][:, c0:c0 + cw],
                                      in_=zbuf[:, c0:c0 + cw])
            if t + 1 >= depth + 2:
                j = 2 * depth - t
                for s in range(NS):
                    c0 = s * 512
                    cw = min(512, NPC - c0)
                    sk = skp.tile([128, 512], f32, tag="sk")
                    nc.sync.dma_start(out=sk[:, :cw], in_=xs_d[j][:, c0:c0 + cw])
                    nc.vector.tensor_add(zbuf[:, c0:c0 + cw],
                                         zbuf[:, c0:c0 + cw], sk[:, :cw])

            build_table(t)

    nc.compile()
    return nc


# ----------------------------------------------------------------------------
# Entry point
# ----------------------------------------------------------------------------

LAST_INFO = {}


def _run(inputs, cfg):
    from concourse.bass_utils import run_bass_kernel_spmd

    x = np.asarray(inputs["x"], np.float32)
    d_in = x.shape[1]
    t0 = time.time()
    shard_maps, plan, nchk = preprocess(x, inputs["edge_index"], cfg)
    t1 = time.time()
    nc = build_nc(cfg, plan, nchk, d_in)
    t2 = time.time()

    common = {
        "W0": np.asarray(inputs["W0"], np.float32),
        "Ws1": np.asarray(inputs["Ws1"], np.float16),
        "Ws2": np.asarray(inputs["Ws2"], np.float16),
        "Wout": np.asarray(inputs["Wout"], np.float16),
        "g1T": np.ascontiguousarray(np.asarray(inputs["g1"], np.float32).T),
        "b1T": np.ascontiguousarray(np.asarray(inputs["b1"], np.float32).T),
        "g2T": np.ascontiguousarray(np.asarray(inputs["g2"], np.float32).T),
        "b2T": np.ascontiguousarray(np.asarray(inputs["b2"], np.float32).T),
        "ident": np.eye(128, dtype=np.float32),
    }
    in_maps = [dict(m, **common) for m in shard_maps]
    kw = {}
    if os.environ.get("KBENCH_TRACE"):
        kw = dict(trace=True, tmpdir=os.environ.get("KBENCH_TMPDIR") or None)
    res = run_bass_kernel_spmd(nc, in_maps, list(range(CORES)), **kw)
    t3 = time.time()
    LAST_INFO.update(preprocess_s=t1 - t0, build_s=t2 - t1, run_s=t3 - t2,
                     exec_time_ns=res.exec_time_ns, nchk=nchk)
    out = np.concatenate(
        [res.results[k]["out"][0, :cfg.S] for k in range(CORES)])
    return out.reshape(cfg.N, 1).astype(np.float32)


def kernel(**inputs):
    return _run(inputs, FULL)
